# revision 15
# baseline (speedup 1.0000x reference)
"""Trainium2 Bass kernel for 16-head RoPE self-attention (S=2048, B=2, D=2048).

Sharding: 8 cores = 2 batches x 4 head-groups (4 heads each). Each core
computes qkv projection for its batch/heads, full attention over its 4
heads, and a partial output projection (its 4-head slice of Wo rows).
Host sums the 4 partial outputs per batch. No cross-core collectives.

v2 schedule (vs v1): the kernel is PE-bound (~92% tensor busy in v1), so
every change targets PE busy-cycles or PE idle gaps:
  - warmup block: ~36 dummy 128-row matmuls at kernel start keep the PE
    HAM activity window busy during the initial weight/x DMA, so the
    clock gate opens at ~3.5us instead of ~25us.
  - qkv projection runs k-tile-outer x head-inner so the first chunk's
    accumulation consumes weight/x tiles in DMA arrival order.
  - RoPE's rotate_half is done with partition-shifted DVE multiplies
    (reading acc straight from PSUM via a bf16 copy) instead of a
    128x128 permutation matmul: -16k PE cycles.
  - softmax denominator: full pairwise DVE tree to one tile, then one
    short ones-matmul per half (issued inside the P item so the PE never
    waits on the exp->tree chain): -25k PE cycles vs v1.
  - phase 2 is a single software-pipelined worklist: scores(i) issue two
    heads ahead of PV(i) so the ScalarE exp backlog never stalls the PE;
    the 4th x-chunk's v-projection and the previous stripe's output
    projection pieces fill the remaining gaps.
  - output is written bf16 (host upcasts + sums partials): halves the
    out-DMA so the tail oproj isn't DMA-limited.
"""

import os
import numpy as np
import ml_dtypes

S, B, D = 2048, 2, 2048
N_HEADS, DQK = 16, 128
ROPE_THETA = 500000.0
N_CORES = 8
CORES_PER_BATCH = 4
NH_LOC = N_HEADS // CORES_PER_BATCH  # 4 heads per core

LAST_RESULT = None  # BassKernelResults of the most recent run (for test.py)

_NC_CACHE = {}


def _build_nc(s, dmodel, nh_loc, d=DQK, sc=512):
    import concourse.tile as tile
    from concourse import bacc, mybir

    bf16 = mybir.dt.bfloat16
    f32 = mybir.dt.float32
    nk = dmodel // 128      # contraction tiles for the projections
    ns = s // 128           # sequence tiles (key side)
    nch = s // sc           # sequence chunks (query side / moving dim)
    hd = nh_loc * d         # local head-dim total (512)
    ne = dmodel // 128      # output-embedding tiles
    sw = 2 * sc             # query stripe width (1024)
    nst = s // sw           # stripes (2)
    h2 = d // 2             # rotate_half split (64)

    nc = bacc.Bacc("TRN2", target_bir_lowering=False, debug=False)
    # inputs are pre-swizzled on the host to partition-major layouts so DMA
    # descriptors are 4-16KB contiguous runs instead of 1KB
    xTq = nc.dram_tensor("xTq", [nch, 128, nk, sc], bf16, kind="ExternalInput")
    wq = nc.dram_tensor("wq", [128, nk, hd], bf16, kind="ExternalInput")
    wk = nc.dram_tensor("wk", [128, nk, hd], bf16, kind="ExternalInput")
    wv = nc.dram_tensor("wv", [128, nk, hd], bf16, kind="ExternalInput")
    wo = nc.dram_tensor("wo", [128, hd // 128, dmodel], bf16,
                        kind="ExternalInput")
    cosT = nc.dram_tensor("cosT", [d, s], bf16, kind="ExternalInput")
    # sshT = sin.T with the BOTTOM 64 rows negated; the multiply for output
    # rows [0:64) reads ssh rows [64:128) (= -sin, base-partition-aligned
    # with raw[64:128)) and vice versa: rot(q)[p]*sin[p] = raw[p^64]*ssh[p^64]
    sshT = nc.dram_tensor("sshT", [d, s], bf16, kind="ExternalInput")
    maskb = nc.dram_tensor("maskb", [128, ns], f32, kind="ExternalInput")
    outT = nc.dram_tensor("outT", [dmodel, s], bf16, kind="ExternalOutput")

    outT_r = outT.rearrange("(e p) s -> p e s", p=128)

    with tile.TileContext(nc) as tc:
        with tc.tile_pool(name="const", bufs=1) as constp, \
             tc.tile_pool(name="store", bufs=1) as storep, \
             tc.tile_pool(name="psA", bufs=2, space="PSUM") as psA:
            qT_sb = storep.tile([128, nh_loc, s], bf16)
            kT_sb = storep.tile([128, nh_loc, s], bf16)
            v_sb = storep.tile([128, ns, hd], bf16)
            attn_sb = storep.tile([128, nh_loc, s], bf16)
            xc3 = storep.tile([128, nk, sc], bf16)  # chunk-3 x, used in ph2

            # ---- phase 1: q,k projection + rope; v for chunks 0-2 --------
            with tc.tile_pool(name="wqk", bufs=1) as wp, \
                 tc.tile_pool(name="tab", bufs=1) as tabp, \
                 tc.tile_pool(name="xch", bufs=2) as xp, \
                 tc.tile_pool(name="rope", bufs=2) as rp, \
                 tc.tile_pool(name="ps1", bufs=6, space="PSUM") as ps1:
                # per-k-tile DMAs in exactly the order the k-outer
                # accumulation consumes them
                wq_sb = wp.tile([128, nk, hd], bf16, tag="wq")
                wk_sb = wp.tile([128, nk, hd], bf16, tag="wk")
                xc0 = xp.tile([128, nk, sc], bf16, tag="xc")
                for k in range(0, nk, 2):
                    ks = slice(k, k + 2)
                    nc.sync.dma_start(wq_sb[:, ks, :], wq[:, ks, :])
                    nc.sync.dma_start(xc0[:, ks, :], xTq[0, :, ks, :])
                for k in range(0, nk, 4):
                    ks = slice(k, k + 4)
                    nc.sync.dma_start(wk_sb[:, ks, :], wk[:, ks, :])
                cos_sb = tabp.tile([128, s], bf16)
                nc.sync.dma_start(cos_sb[:], cosT[:])
                ssh_sb = tabp.tile([128, s], bf16)
                nc.sync.dma_start(ssh_sb[:], sshT[:])
                maskb_sb = constp.tile([128, ns], f32)
                nc.sync.dma_start(maskb_sb[:], maskb[:])
                wv_sb = constp.tile([128, nk, hd], bf16)
                gw = 4
                for g in range(0, nk, gw):
                    gs = slice(g, g + gw)
                    nc.sync.dma_start(wv_sb[:, gs, :], wv[:, gs, :])
                # chunk-1 x before wo: chunk 1's first matmul needs it ~45us
                # in, wo isn't needed until ~150us
                xcs = {0: xc0}
                xcs[1] = xp.tile([128, nk, sc], bf16, tag="xc", name="xc1")
                nc.sync.dma_start(xcs[1][:], xTq[1])
                wo_sb = constp.tile([128, nh_loc, dmodel], bf16)
                nc.sync.dma_start(wo_sb[:], wo[:])

                # warmup: keep the PE busy while the first DMAs land so the
                # HAM clock gate opens early.  The operands come from a memset
                # (no DMA dependency -> starts at ~0.5us); 512-row matmuls
                # keep the PE duty cycle high (the paired LDWEIGHTS is only
                # 128 rows), round-robining 4 PSUM slots to avoid WAW stalls.
                # ones_sb doubles as the denominator's summing stationary.
                ones_sb = constp.tile([128, sc], bf16)
                nc.vector.memset(ones_sb[:], 1.0)
                warms = []
                for _ in range(4):
                    warm = ps1.tile([128, sc], f32, tag="acc", name="warm")
                    warms.append(warm)
                for w in range(16):
                    warm = warms[w % 4]
                    nc.tensor.matmul(warm[:], ones_sb[:, :128], ones_sb[:],
                                     start=True, stop=True)

                def rope(acc, dstT, h, csl):
                    raw = rp.tile([128, sc], bf16, tag="raw")
                    nc.scalar.copy(raw[:], acc[:])
                    t1 = rp.tile([128, sc], bf16, tag="t1")
                    nc.vector.tensor_mul(t1[:], raw[:], cos_sb[:, csl])
                    t2 = rp.tile([128, sc], bf16, tag="t2")
                    # both SBUF inputs of a TensorTensor must share a base
                    # partition, so each multiply uses the ssh rows aligned
                    # with its raw slice (sin rows repeat: sin[m]==sin[m+64])
                    nc.vector.tensor_mul(t2[:h2, :], raw[h2:, :],
                                         ssh_sb[h2:, csl])
                    nc.vector.tensor_mul(t2[h2:, :], raw[:h2, :],
                                         ssh_sb[:h2, csl])
                    nc.vector.tensor_add(dstT[:, h, csl], t1[:], t2[:])

                for ch in range(nch):
                    csl = slice(ch * sc, (ch + 1) * sc)
                    # prefetch next chunk's x (chunks 0,1 pre-issued above)
                    nxt = ch + 1
                    if nxt < nch and nxt not in xcs:
                        if nxt == nch - 1:
                            xcs[nxt] = xc3
                        else:
                            xcs[nxt] = xp.tile([128, nk, sc], bf16, tag="xc",
                                               name="xcn")
                        nc.sync.dma_start(xcs[nxt][:], xTq[nxt])
                    xc = xcs[ch]
                    # q,k accumulation, k-tile-outer.  Last chunk does k
                    # heads first so kT (needed by the first scores) clears
                    # the rope pipeline early.
                    tlist = (("q", wq_sb, qT_sb), ("k", wk_sb, kT_sb))
                    if ch == nch - 1:
                        tlist = tlist[::-1]
                    for t, w_sb, dstT in tlist:
                        accs = []
                        for h in range(nh_loc):
                            acc = ps1.tile([128, sc], f32, tag="acc")
                            accs.append((acc, h))
                        for k in range(nk):
                            for acc, h in accs:
                                hsl = slice(h * d, (h + 1) * d)
                                nc.tensor.matmul(
                                    acc[:], w_sb[:, k, hsl], xc[:, k, :],
                                    start=(k == 0), stop=(k == nk - 1),
                                )
                        for acc, h in accs:
                            rope(acc, dstT, h, csl)
                    # v for chunks 0-2 (chunk 3 is deferred into phase 2)
                    if ch < nch - 1:
                        for stl in range(sc // 128):
                            st = ch * (sc // 128) + stl
                            ssl = slice(stl * 128, (stl + 1) * 128)
                            accv = ps1.tile([128, hd], f32, tag="acc")
                            for k in range(nk):
                                nc.tensor.matmul(
                                    accv[:], xc[:, k, ssl], wv_sb[:, k, :],
                                    start=(k == 0), stop=(k == nk - 1),
                                )
                            nc.scalar.copy(v_sb[:, st, :], accv[:])

            # ---- phase 2: attention + output projection ------------------
            with tc.tile_pool(name="expp", bufs=2) as expp, \
                 tc.tile_pool(name="tree", bufs=8) as treep, \
                 tc.tile_pool(name="invp", bufs=1) as invp, \
                 tc.tile_pool(name="outp", bufs=4) as outp, \
                 tc.tile_pool(name="ps2s", bufs=2, space="PSUM") as ps2s, \
                 tc.tile_pool(name="ps2p", bufs=2, space="PSUM") as ps2p:

                exs = {}    # i -> exp tile
                roots = {}  # i -> denominator tree root (SBUF bf16)

                def item_S(i):
                    ic, h = divmod(i, nh_loc)
                    ex = expp.tile([128, ns, sw], bf16, tag="exp")
                    exs[i] = ex
                    for jt in range(ns):
                        # weave an oproj unit in every 4 j-tiles: when the
                        # exp backlog paces the scores, this keeps PE busy
                        if jt in (4, 8, 12) and ofill:
                            o_unit()
                        jsl = slice(jt * 128, (jt + 1) * 128)
                        sc_ps = ps2s.tile([128, sw], f32, tag="scores")
                        for half in range(2):
                            qsl = slice(ic * sw + half * sc,
                                        ic * sw + (half + 1) * sc)
                            nc.tensor.matmul(
                                sc_ps[:, half * sc:(half + 1) * sc],
                                kT_sb[:, h, jsl], qT_sb[:, h, qsl],
                                start=True, stop=True)
                        nc.scalar.activation(
                            ex[:, jt, :], sc_ps[:],
                            mybir.ActivationFunctionType.Exp,
                            bias=maskb_sb[:, jt:jt + 1], scale=1.0,
                        )
                    # denominator part 1: pairwise DVE tree to one tile
                    # (the j-tile sum; starts as soon as exp pairs land)
                    us = []
                    for a in range(ns // 2):
                        u = treep.tile([128, sw], bf16, tag="tree")
                        nc.vector.tensor_add(u[:], ex[:, 2 * a, :],
                                             ex[:, 2 * a + 1, :])
                        us.append(u)
                    step = 1
                    while step < len(us):
                        for b in range(0, len(us), 2 * step):
                            nc.vector.tensor_add(us[b][:], us[b][:],
                                                 us[b + step][:])
                        step *= 2
                    roots[i] = us[0]

                def item_P(i):
                    ic, h = divmod(i, nh_loc)
                    hsl = slice(h * d, (h + 1) * d)
                    ex = exs.pop(i)
                    root = roots.pop(i)
                    # denominator part 2: sum the 128 j-partitions with one
                    # short ones-matmul per half.  Issued here (not in S)
                    # so the PE doesn't sit on the exp->tree dependency.
                    inv = invp.tile([128, sw], f32, tag="inv")
                    rps = []
                    for half in range(2):
                        fsl = slice(half * sc, (half + 1) * sc)
                        r_ps = psA.tile([128, sc], f32, tag="oproj",
                                         name="r_ps")
                        nc.tensor.matmul(r_ps[:], ones_sb[:, :128], root[:, fsl],
                                         start=True, stop=True)
                        rps.append((r_ps, fsl))
                    for r_ps, fsl in rps:
                        nc.vector.reciprocal_approx_fast(inv[:, fsl], r_ps[:])
                    for half in range(2):
                        fsl = slice(half * sc, (half + 1) * sc)
                        pv_ps = ps2p.tile([128, sc], f32, tag="pv")
                        for jt in range(ns):
                            nc.tensor.matmul(
                                pv_ps[:], v_sb[:, jt, hsl], ex[:, jt, fsl],
                                start=(jt == 0), stop=(jt == ns - 1),
                            )
                        asl = slice(ic * sw + half * sc,
                                    ic * sw + (half + 1) * sc)
                        nc.vector.tensor_mul(attn_sb[:, h, asl],
                                             pv_ps[:], inv[:, fsl])

                def item_V(st):
                    stl = st - (nch - 1) * (sc // 128)
                    ssl = slice(stl * 128, (stl + 1) * 128)
                    accv = psA.tile([128, hd], f32, tag="oproj")
                    for k in range(nk):
                        nc.tensor.matmul(
                            accv[:], xc3[:, k, ssl], wv_sb[:, k, :],
                            start=(k == 0), stop=(k == nk - 1),
                        )
                    nc.scalar.copy(v_sb[:, st, :], accv[:])

                # output projection, issued as (et, half) units that fill
                # PE slack between (and inside) the S/P items
                ofill = []
                ocnt = [0]

                def o_unit():
                    ic, et, half = ofill.pop(0)
                    esl = slice(et * 128, (et + 1) * 128)
                    osl = slice(ic * sw + half * sc,
                                ic * sw + (half + 1) * sc)
                    op_ps = psA.tile([128, sc], f32, tag="oproj")
                    for ht in range(nh_loc):
                        nc.tensor.matmul(
                            op_ps[:], wo_sb[:, ht, esl],
                            attn_sb[:, ht, osl],
                            start=(ht == 0), stop=(ht == nh_loc - 1),
                        )
                    ot = outp.tile([128, sc], bf16, tag="ot")
                    # stripe-0 units run while ScalarE is saturated with exp:
                    # drain on DVE only.  Tail units alternate DVE/ScalarE.
                    ocnt[0] += 1
                    if ic == 0 or ocnt[0] % 2 == 0:
                        nc.vector.tensor_copy(ot[:], op_ps[:])
                    else:
                        nc.scalar.copy(ot[:], op_ps[:])
                    nc.sync.dma_start(outT_r[:, et, osl], ot[:])

                v3 = (nch - 1) * (sc // 128)
                sched = [("V", v3), ("V", v3 + 1), ("V", v3 + 2),
                         ("S", 0), ("S", 1), ("V", v3 + 3),
                         ("P", 0), ("S", 2), ("P", 1),
                         ("S", 3), ("P", 2),
                         ("S", 4), ("P", 3), ("U", 4), ("S", 5), ("U", 2),
                         ("P", 4), ("U", 4), ("S", 6), ("U", 2), ("P", 5),
                         ("U", 4), ("S", 7), ("U", 2), ("P", 6), ("U", 5),
                         ("P", 7)]
                assert nst == 2 and nh_loc == 4
                for kind, arg in sched:
                    if kind == "S":
                        item_S(arg)
                    elif kind == "P":
                        item_P(arg)
                        i = arg
                        if i % nh_loc == nh_loc - 1:
                            ic = i // nh_loc
                            ofill.extend((ic, et, half) for et in range(ne)
                                         for half in range(2))
                    elif kind == "V":
                        item_V(arg)
                    else:
                        for _ in range(min(arg, len(ofill))):
                            o_unit()
                while ofill:
                    o_unit()

    nc.compile()
    return nc


def _get_nc(s=S, dmodel=D, nh_loc=NH_LOC):
    key = (s, dmodel, nh_loc)
    if key not in _NC_CACHE:
        _NC_CACHE[key] = _build_nc(s, dmodel, nh_loc)
    return _NC_CACHE[key]


def _rope_tables(s, d, dtype=np.float32):
    inv_freq = 1.0 / (ROPE_THETA ** (np.arange(0, d, 2, dtype=np.float64) / d))
    pos = np.arange(s, dtype=np.float64)
    freqs = pos[:, None] * inv_freq[None, :]            # [s, d/2]
    emb = np.concatenate([freqs, freqs], axis=-1)       # [s, d]
    return np.cos(emb).astype(dtype), np.sin(emb).astype(dtype)


def make_in_maps(hidden_states, sequence_mask, Wqkv, Wo,
                 s=S, b=B, dmodel=D, nh_tot=N_HEADS, nh_loc=NH_LOC, d=DQK):
    bf = ml_dtypes.bfloat16
    cos, sin = _rope_tables(s, d)
    cosT = np.ascontiguousarray(cos.T).astype(bf)       # [d, s]
    sinT = np.ascontiguousarray(sin.T)                  # [d, s] f32
    ssh = sinT.copy()
    ssh[d // 2:] = -ssh[d // 2:]
    sshT = ssh.astype(bf)
    scale = 1.0 / np.sqrt(np.float32(d))

    in_maps = []
    cores_per_batch = N_CORES // b
    for c in range(N_CORES):
        bi = c // cores_per_batch
        g = c % cores_per_batch
        h0 = g * nh_loc
        hsl = slice(h0 * d, (h0 + nh_loc) * d)
        nk, sc, nch = dmodel // 128, 512, s // 512
        xb = hidden_states[:, bi, :]                    # [s, dmodel]
        # [nch, 128, nk, sc]: xTq[ch, p, k, j] = x[ch*sc+j, k*128+p]
        xTq = np.ascontiguousarray(
            xb.T.reshape(nk, 128, nch, sc).transpose(2, 1, 0, 3)).astype(bf)
        hd = nh_loc * d

        def swz_w(w):   # [dmodel, hd] -> [128, nk, hd]
            return np.ascontiguousarray(
                w.reshape(nk, 128, hd).transpose(1, 0, 2)).astype(bf)

        wq = swz_w(Wqkv[:, 0 * nh_tot * d:1 * nh_tot * d][:, hsl] * scale)
        wk = swz_w(Wqkv[:, 1 * nh_tot * d:2 * nh_tot * d][:, hsl])
        wv = swz_w(Wqkv[:, 2 * nh_tot * d:3 * nh_tot * d][:, hsl])
        # [128, nh_loc, dmodel]: wo[p, h, e] = Wo[h*128+p, e]
        wo = np.ascontiguousarray(
            Wo[hsl, :].reshape(nh_loc, 128, dmodel).transpose(1, 0, 2)
        ).astype(bf)
        bias = np.where(sequence_mask[bi] == 0, -1e30, 0.0).astype(np.float32)
        maskbT = np.ascontiguousarray(bias.reshape(s // 128, 128).T)  # [128, ns]
        in_maps.append({
            "xTq": xTq, "wq": wq, "wk": wk, "wv": wv, "wo": wo,
            "cosT": cosT, "sshT": sshT, "maskb": maskbT,
        })
    return in_maps


def kernel(hidden_states, sequence_mask, Wqkv, Wo):
    global LAST_RESULT
    from concourse.bass_utils import run_bass_kernel_spmd

    hidden_states = np.asarray(hidden_states)
    sequence_mask = np.asarray(sequence_mask)
    Wqkv = np.asarray(Wqkv)
    Wo = np.asarray(Wo)

    nc = _get_nc()
    in_maps = make_in_maps(hidden_states, sequence_mask, Wqkv, Wo)
    res = run_bass_kernel_spmd(
        nc, in_maps, list(range(N_CORES)),
        trace=bool(int(os.environ.get("KERNEL_TRACE", "0"))),
    )
    LAST_RESULT = res

    out = np.empty((S, B, D), dtype=np.float32)
    cores_per_batch = N_CORES // B
    for bi in range(B):
        acc = None
        for g in range(cores_per_batch):
            part = res.results[bi * cores_per_batch + g]["outT"]  # [D, S] bf16
            part = np.asarray(part, dtype=np.float32)
            acc = part if acc is None else acc + part
        out[:, bi, :] = acc.T
    return out


# revision 17
# speedup vs baseline: 1.0150x; 1.0150x over previous
"""Trainium2 Bass kernel for 16-head RoPE self-attention (S=2048, B=2, D=2048).

Sharding: 8 cores = 2 batches x 4 head-groups (4 heads each). Each core
computes qkv projection for its batch/heads, full attention over its 4
heads, and a partial output projection (its 4-head slice of Wo rows).
Host sums the 4 partial outputs per batch. No cross-core collectives.

v2 schedule (vs v1): the kernel is PE-bound (~92% tensor busy in v1), so
every change targets PE busy-cycles or PE idle gaps:
  - warmup block: ~36 dummy 128-row matmuls at kernel start keep the PE
    HAM activity window busy during the initial weight/x DMA, so the
    clock gate opens at ~3.5us instead of ~25us.
  - qkv projection runs k-tile-outer x head-inner so the first chunk's
    accumulation consumes weight/x tiles in DMA arrival order.
  - RoPE's rotate_half is done with partition-shifted DVE multiplies
    (reading acc straight from PSUM via a bf16 copy) instead of a
    128x128 permutation matmul: -16k PE cycles.
  - softmax denominator: full pairwise DVE tree to one tile, then one
    short ones-matmul per half (issued inside the P item so the PE never
    waits on the exp->tree chain): -25k PE cycles vs v1.
  - phase 2 is a single software-pipelined worklist: scores(i) issue two
    heads ahead of PV(i) so the ScalarE exp backlog never stalls the PE;
    the 4th x-chunk's v-projection and the previous stripe's output
    projection pieces fill the remaining gaps.
  - output is written bf16 (host upcasts + sums partials): halves the
    out-DMA so the tail oproj isn't DMA-limited.
"""

import os
import numpy as np
import ml_dtypes

S, B, D = 2048, 2, 2048
N_HEADS, DQK = 16, 128
ROPE_THETA = 500000.0
N_CORES = 8
CORES_PER_BATCH = 4
NH_LOC = N_HEADS // CORES_PER_BATCH  # 4 heads per core

LAST_RESULT = None  # BassKernelResults of the most recent run (for test.py)

_NC_CACHE = {}


def _build_nc(s, dmodel, nh_loc, d=DQK, sc=512):
    import concourse.tile as tile
    from concourse import bacc, mybir

    bf16 = mybir.dt.bfloat16
    f32 = mybir.dt.float32
    nk = dmodel // 128      # contraction tiles for the projections
    ns = s // 128           # sequence tiles (key side)
    nch = s // sc           # sequence chunks (query side / moving dim)
    hd = nh_loc * d         # local head-dim total (512)
    ne = dmodel // 128      # output-embedding tiles
    sw = 2 * sc             # query stripe width (1024)
    nst = s // sw           # stripes (2)
    h2 = d // 2             # rotate_half split (64)

    nc = bacc.Bacc("TRN2", target_bir_lowering=False, debug=False)
    # inputs are pre-swizzled on the host to partition-major layouts so DMA
    # descriptors are 4-16KB contiguous runs instead of 1KB
    xTq = nc.dram_tensor("xTq", [nch, 128, nk, sc], bf16, kind="ExternalInput")
    wq = nc.dram_tensor("wq", [128, nk, hd], bf16, kind="ExternalInput")
    wk = nc.dram_tensor("wk", [128, nk, hd], bf16, kind="ExternalInput")
    wv = nc.dram_tensor("wv", [128, nk, hd], bf16, kind="ExternalInput")
    wo = nc.dram_tensor("wo", [128, hd // 128, dmodel], bf16,
                        kind="ExternalInput")
    cosT = nc.dram_tensor("cosT", [d, s], bf16, kind="ExternalInput")
    # sshT = sin.T with the BOTTOM 64 rows negated; the multiply for output
    # rows [0:64) reads ssh rows [64:128) (= -sin, base-partition-aligned
    # with raw[64:128)) and vice versa: rot(q)[p]*sin[p] = raw[p^64]*ssh[p^64]
    sshT = nc.dram_tensor("sshT", [d, s], bf16, kind="ExternalInput")
    maskb = nc.dram_tensor("maskb", [128, ns], f32, kind="ExternalInput")
    outT = nc.dram_tensor("outT", [dmodel, s], bf16, kind="ExternalOutput")

    outT_r = outT.rearrange("(e p) s -> p e s", p=128)

    with tile.TileContext(nc) as tc:
        with tc.tile_pool(name="const", bufs=1) as constp, \
             tc.tile_pool(name="store", bufs=1) as storep, \
             tc.tile_pool(name="psA", bufs=2, space="PSUM") as psA:
            qT_sb = storep.tile([128, nh_loc, s], bf16)
            kT_sb = storep.tile([128, nh_loc, s], bf16)
            v_sb = storep.tile([128, ns, hd], bf16)
            attn_sb = storep.tile([128, nh_loc, s], bf16)
            xc3 = storep.tile([128, nk, sc], bf16)  # chunk-3 x, used in ph2

            # ---- phase 1: q,k projection + rope; v for chunks 0-2 --------
            with tc.tile_pool(name="wqk", bufs=1) as wp, \
                 tc.tile_pool(name="tab", bufs=1) as tabp, \
                 tc.tile_pool(name="xch", bufs=2) as xp, \
                 tc.tile_pool(name="rope", bufs=2) as rp, \
                 tc.tile_pool(name="ps1", bufs=6, space="PSUM") as ps1:
                # per-k-tile DMAs in exactly the order the k-outer
                # accumulation consumes them
                wq_sb = wp.tile([128, nk, hd], bf16, tag="wq")
                wk_sb = wp.tile([128, nk, hd], bf16, tag="wk")
                xc0 = xp.tile([128, nk, sc], bf16, tag="xc")
                for k in range(0, nk, 2):
                    ks = slice(k, k + 2)
                    nc.sync.dma_start(wq_sb[:, ks, :], wq[:, ks, :])
                    nc.sync.dma_start(xc0[:, ks, :], xTq[0, :, ks, :])
                for k in range(0, nk, 4):
                    ks = slice(k, k + 4)
                    nc.sync.dma_start(wk_sb[:, ks, :], wk[:, ks, :])
                cos_sb = tabp.tile([128, s], bf16)
                nc.sync.dma_start(cos_sb[:], cosT[:])
                ssh_sb = tabp.tile([128, s], bf16)
                nc.sync.dma_start(ssh_sb[:], sshT[:])
                maskb_sb = constp.tile([128, ns], f32)
                nc.sync.dma_start(maskb_sb[:], maskb[:])
                wv_sb = constp.tile([128, nk, hd], bf16)
                gw = 4
                for g in range(0, nk, gw):
                    gs = slice(g, g + gw)
                    nc.sync.dma_start(wv_sb[:, gs, :], wv[:, gs, :])
                # chunk-1 x before wo: chunk 1's first matmul needs it ~45us
                # in, wo isn't needed until ~150us
                xcs = {0: xc0}
                xcs[1] = xp.tile([128, nk, sc], bf16, tag="xc", name="xc1")
                nc.sync.dma_start(xcs[1][:], xTq[1])
                wo_sb = constp.tile([128, nh_loc, dmodel], bf16)
                nc.sync.dma_start(wo_sb[:], wo[:])

                # warmup: keep the PE busy while the first DMAs land so the
                # HAM clock gate opens early.  The operands come from a memset
                # (no DMA dependency -> starts at ~0.5us); 512-row matmuls
                # keep the PE duty cycle high (the paired LDWEIGHTS is only
                # 128 rows), round-robining 4 PSUM slots to avoid WAW stalls.
                # ones_sb doubles as the denominator's summing stationary.
                ones_sb = constp.tile([128, sc], bf16)
                nc.vector.memset(ones_sb[:], 1.0)
                warms = []
                for _ in range(4):
                    warm = ps1.tile([128, sc], f32, tag="acc", name="warm")
                    warms.append(warm)
                for w in range(16):
                    warm = warms[w % 4]
                    nc.tensor.matmul(warm[:], ones_sb[:, :128], ones_sb[:],
                                     start=True, stop=True)

                def rope(acc, dstT, h, csl):
                    raw = rp.tile([128, sc], bf16, tag="raw")
                    nc.scalar.copy(raw[:], acc[:])
                    t1 = rp.tile([128, sc], bf16, tag="t1")
                    nc.vector.tensor_mul(t1[:], raw[:], cos_sb[:, csl])
                    t2 = rp.tile([128, sc], bf16, tag="t2")
                    # both SBUF inputs of a TensorTensor must share a base
                    # partition, so each multiply uses the ssh rows aligned
                    # with its raw slice (sin rows repeat: sin[m]==sin[m+64])
                    nc.vector.tensor_mul(t2[:h2, :], raw[h2:, :],
                                         ssh_sb[h2:, csl])
                    nc.vector.tensor_mul(t2[h2:, :], raw[:h2, :],
                                         ssh_sb[:h2, csl])
                    nc.vector.tensor_add(dstT[:, h, csl], t1[:], t2[:])

                for ch in range(nch):
                    csl = slice(ch * sc, (ch + 1) * sc)
                    # prefetch next chunk's x (chunks 0,1 pre-issued above)
                    nxt = ch + 1
                    if nxt < nch and nxt not in xcs:
                        if nxt == nch - 1:
                            xcs[nxt] = xc3
                        else:
                            xcs[nxt] = xp.tile([128, nk, sc], bf16, tag="xc",
                                               name="xcn")
                        nc.sync.dma_start(xcs[nxt][:], xTq[nxt])
                    xc = xcs[ch]
                    # q,k accumulation, k-tile-outer.  Last chunk does k
                    # heads first so kT (needed by the first scores) clears
                    # the rope pipeline early.
                    tlist = (("q", wq_sb, qT_sb), ("k", wk_sb, kT_sb))
                    if ch == nch - 1:
                        tlist = tlist[::-1]
                    for t, w_sb, dstT in tlist:
                        accs = []
                        for h in range(nh_loc):
                            acc = ps1.tile([128, sc], f32, tag="acc")
                            accs.append((acc, h))
                        for k in range(nk):
                            for acc, h in accs:
                                hsl = slice(h * d, (h + 1) * d)
                                nc.tensor.matmul(
                                    acc[:], w_sb[:, k, hsl], xc[:, k, :],
                                    start=(k == 0), stop=(k == nk - 1),
                                )
                        for acc, h in accs:
                            rope(acc, dstT, h, csl)
                    # v for chunks 0-2 (chunk 3 is deferred into phase 2)
                    if ch < nch - 1:
                        for stl in range(sc // 128):
                            st = ch * (sc // 128) + stl
                            ssl = slice(stl * 128, (stl + 1) * 128)
                            accv = ps1.tile([128, hd], f32, tag="acc")
                            for k in range(nk):
                                nc.tensor.matmul(
                                    accv[:], xc[:, k, ssl], wv_sb[:, k, :],
                                    start=(k == 0), stop=(k == nk - 1),
                                )
                            nc.scalar.copy(v_sb[:, st, :], accv[:])

            # ---- phase 2: attention + output projection ------------------
            with tc.tile_pool(name="expp", bufs=2) as expp, \
                 tc.tile_pool(name="tree", bufs=2) as treep, \
                 tc.tile_pool(name="invp", bufs=1) as invp, \
                 tc.tile_pool(name="outp", bufs=4) as outp, \
                 tc.tile_pool(name="ps2s", bufs=2, space="PSUM") as ps2s, \
                 tc.tile_pool(name="ps2p", bufs=2, space="PSUM") as ps2p:

                exs = {}    # i -> exp tile
                roots = {}  # i -> denominator tree root (SBUF bf16)

                def item_S(i):
                    ic, h = divmod(i, nh_loc)
                    ex = expp.tile([128, ns, sw], bf16, tag="exp")
                    exs[i] = ex
                    for jt in range(ns):
                        # weave an oproj unit in every 4 j-tiles: when the
                        # exp backlog paces the scores, this keeps PE busy
                        if jt in (4, 8, 12) and ofill:
                            o_unit()
                        jsl = slice(jt * 128, (jt + 1) * 128)
                        sc_ps = ps2s.tile([128, sw], f32, tag="scores")
                        for half in range(2):
                            qsl = slice(ic * sw + half * sc,
                                        ic * sw + (half + 1) * sc)
                            nc.tensor.matmul(
                                sc_ps[:, half * sc:(half + 1) * sc],
                                kT_sb[:, h, jsl], qT_sb[:, h, qsl],
                                start=True, stop=True)
                        nc.scalar.activation(
                            ex[:, jt, :], sc_ps[:],
                            mybir.ActivationFunctionType.Exp,
                            bias=maskb_sb[:, jt:jt + 1], scale=1.0,
                        )
                    # denominator part 1: running j-tile sum into one tile
                    # (DVE, paced by the exps as they land)
                    u = treep.tile([128, sw], bf16, tag="tree")
                    nc.vector.tensor_add(u[:], ex[:, 0, :], ex[:, 1, :])
                    for a in range(2, ns):
                        nc.vector.tensor_add(u[:], u[:], ex[:, a, :])
                    roots[i] = u

                def item_P(i):
                    ic, h = divmod(i, nh_loc)
                    hsl = slice(h * d, (h + 1) * d)
                    ex = exs.pop(i)
                    root = roots.pop(i)
                    # denominator part 2: sum the 128 j-partitions with one
                    # short ones-matmul per half.  Issued here (not in S)
                    # so the PE doesn't sit on the exp->tree dependency.
                    inv = invp.tile([128, sw], f32, tag="inv")
                    rps = []
                    for half in range(2):
                        fsl = slice(half * sc, (half + 1) * sc)
                        r_ps = psA.tile([128, sc], f32, tag="oproj",
                                         name="r_ps")
                        nc.tensor.matmul(r_ps[:], ones_sb[:, :128], root[:, fsl],
                                         start=True, stop=True)
                        rps.append((r_ps, fsl))
                    for r_ps, fsl in rps:
                        nc.vector.reciprocal_approx_fast(inv[:, fsl], r_ps[:])
                    for half in range(2):
                        fsl = slice(half * sc, (half + 1) * sc)
                        pv_ps = ps2p.tile([128, sc], f32, tag="pv")
                        for jt in range(ns):
                            nc.tensor.matmul(
                                pv_ps[:], v_sb[:, jt, hsl], ex[:, jt, fsl],
                                start=(jt == 0), stop=(jt == ns - 1),
                            )
                        asl = slice(ic * sw + half * sc,
                                    ic * sw + (half + 1) * sc)
                        nc.vector.tensor_mul(attn_sb[:, h, asl],
                                             pv_ps[:], inv[:, fsl])

                def item_V(st):
                    stl = st % (sc // 128)
                    ssl = slice(stl * 128, (stl + 1) * 128)
                    xcv = xc3
                    accv = psA.tile([128, hd], f32, tag="oproj")
                    for k in range(nk):
                        nc.tensor.matmul(
                            accv[:], xcv[:, k, ssl], wv_sb[:, k, :],
                            start=(k == 0), stop=(k == nk - 1),
                        )
                    nc.scalar.copy(v_sb[:, st, :], accv[:])

                # output projection, issued as (et, half) units that fill
                # PE slack between (and inside) the S/P items
                ofill = []
                ocnt = [0]

                def o_unit():
                    ic, et, half = ofill.pop(0)
                    esl = slice(et * 128, (et + 1) * 128)
                    osl = slice(ic * sw + half * sc,
                                ic * sw + (half + 1) * sc)
                    op_ps = psA.tile([128, sc], f32, tag="oproj")
                    for ht in range(nh_loc):
                        nc.tensor.matmul(
                            op_ps[:], wo_sb[:, ht, esl],
                            attn_sb[:, ht, osl],
                            start=(ht == 0), stop=(ht == nh_loc - 1),
                        )
                    ot = outp.tile([128, sc], bf16, tag="ot")
                    # stripe-0 units run while ScalarE is saturated with exp:
                    # drain on DVE only.  Tail units alternate DVE/ScalarE.
                    ocnt[0] += 1
                    if ic == 0 or ocnt[0] % 2 == 0:
                        nc.vector.tensor_copy(ot[:], op_ps[:])
                    else:
                        nc.scalar.copy(ot[:], op_ps[:])
                    nc.sync.dma_start(outT_r[:, et, osl], ot[:])

                v3 = (nch - 1) * (sc // 128)
                sched = [("V", v3), ("V", v3 + 1), ("V", v3 + 2),
                         ("S", 0), ("S", 1), ("V", v3 + 3),
                         ("P", 0), ("S", 2), ("P", 1),
                         ("S", 3), ("P", 2),
                         ("S", 4), ("P", 3), ("U", 4), ("S", 5), ("U", 2),
                         ("P", 4), ("U", 4), ("S", 6), ("U", 2), ("P", 5),
                         ("U", 4), ("S", 7), ("U", 2), ("P", 6), ("U", 5),
                         ("P", 7)]
                assert nst == 2 and nh_loc == 4
                for kind, arg in sched:
                    if kind == "S":
                        item_S(arg)
                    elif kind == "P":
                        item_P(arg)
                        i = arg
                        if i % nh_loc == nh_loc - 1:
                            ic = i // nh_loc
                            ofill.extend((ic, et, half) for et in range(ne)
                                         for half in range(2))
                    elif kind == "V":
                        item_V(arg)
                    else:
                        for _ in range(min(arg, len(ofill))):
                            o_unit()
                while ofill:
                    o_unit()

    nc.compile()
    return nc


def _get_nc(s=S, dmodel=D, nh_loc=NH_LOC):
    key = (s, dmodel, nh_loc)
    if key not in _NC_CACHE:
        _NC_CACHE[key] = _build_nc(s, dmodel, nh_loc)
    return _NC_CACHE[key]


def _rope_tables(s, d, dtype=np.float32):
    inv_freq = 1.0 / (ROPE_THETA ** (np.arange(0, d, 2, dtype=np.float64) / d))
    pos = np.arange(s, dtype=np.float64)
    freqs = pos[:, None] * inv_freq[None, :]            # [s, d/2]
    emb = np.concatenate([freqs, freqs], axis=-1)       # [s, d]
    return np.cos(emb).astype(dtype), np.sin(emb).astype(dtype)


def make_in_maps(hidden_states, sequence_mask, Wqkv, Wo,
                 s=S, b=B, dmodel=D, nh_tot=N_HEADS, nh_loc=NH_LOC, d=DQK):
    bf = ml_dtypes.bfloat16
    cos, sin = _rope_tables(s, d)
    cosT = np.ascontiguousarray(cos.T).astype(bf)       # [d, s]
    sinT = np.ascontiguousarray(sin.T)                  # [d, s] f32
    ssh = sinT.copy()
    ssh[d // 2:] = -ssh[d // 2:]
    sshT = ssh.astype(bf)
    scale = 1.0 / np.sqrt(np.float32(d))

    in_maps = []
    cores_per_batch = N_CORES // b
    for c in range(N_CORES):
        bi = c // cores_per_batch
        g = c % cores_per_batch
        h0 = g * nh_loc
        hsl = slice(h0 * d, (h0 + nh_loc) * d)
        nk, sc, nch = dmodel // 128, 512, s // 512
        xb = hidden_states[:, bi, :]                    # [s, dmodel]
        # [nch, 128, nk, sc]: xTq[ch, p, k, j] = x[ch*sc+j, k*128+p]
        xTq = np.ascontiguousarray(
            xb.T.reshape(nk, 128, nch, sc).transpose(2, 1, 0, 3)).astype(bf)
        hd = nh_loc * d

        def swz_w(w):   # [dmodel, hd] -> [128, nk, hd]
            return np.ascontiguousarray(
                w.reshape(nk, 128, hd).transpose(1, 0, 2)).astype(bf)

        wq = swz_w(Wqkv[:, 0 * nh_tot * d:1 * nh_tot * d][:, hsl] * scale)
        wk = swz_w(Wqkv[:, 1 * nh_tot * d:2 * nh_tot * d][:, hsl])
        wv = swz_w(Wqkv[:, 2 * nh_tot * d:3 * nh_tot * d][:, hsl])
        # [128, nh_loc, dmodel]: wo[p, h, e] = Wo[h*128+p, e]
        wo = np.ascontiguousarray(
            Wo[hsl, :].reshape(nh_loc, 128, dmodel).transpose(1, 0, 2)
        ).astype(bf)
        bias = np.where(sequence_mask[bi] == 0, -1e30, 0.0).astype(np.float32)
        maskbT = np.ascontiguousarray(bias.reshape(s // 128, 128).T)  # [128, ns]
        in_maps.append({
            "xTq": xTq, "wq": wq, "wk": wk, "wv": wv, "wo": wo,
            "cosT": cosT, "sshT": sshT, "maskb": maskbT,
        })
    return in_maps


def kernel(hidden_states, sequence_mask, Wqkv, Wo):
    global LAST_RESULT
    from concourse.bass_utils import run_bass_kernel_spmd

    hidden_states = np.asarray(hidden_states)
    sequence_mask = np.asarray(sequence_mask)
    Wqkv = np.asarray(Wqkv)
    Wo = np.asarray(Wo)

    nc = _get_nc()
    in_maps = make_in_maps(hidden_states, sequence_mask, Wqkv, Wo)
    res = run_bass_kernel_spmd(
        nc, in_maps, list(range(N_CORES)),
        trace=bool(int(os.environ.get("KERNEL_TRACE", "0"))),
    )
    LAST_RESULT = res

    out = np.empty((S, B, D), dtype=np.float32)
    cores_per_batch = N_CORES // B
    for bi in range(B):
        acc = None
        for g in range(cores_per_batch):
            part = res.results[bi * cores_per_batch + g]["outT"]  # [D, S] bf16
            part = np.asarray(part, dtype=np.float32)
            acc = part if acc is None else acc + part
        out[:, bi, :] = acc.T
    return out


# revision 19
# speedup vs baseline: 1.0171x; 1.0021x over previous
"""Trainium2 Bass kernel for 16-head RoPE self-attention (S=2048, B=2, D=2048).

Sharding: 8 cores = 2 batches x 4 head-groups (4 heads each). Each core
computes qkv projection for its batch/heads, full attention over its 4
heads, and a partial output projection (its 4-head slice of Wo rows).
Host sums the 4 partial outputs per batch. No cross-core collectives.

The kernel is PE-bound (~92% tensor busy), so every change targets PE
busy-cycles or PE idle gaps:
  - warmup block: 16 dummy 512-row matmuls on a memset tile at kernel
    start keep the PE HAM activity window busy during the initial
    weight/x DMA, so the clock gate opens at ~11us instead of ~25us.
  - qkv projection runs k-tile-outer x head-inner so the first chunk's
    accumulation consumes weight/x tiles in DMA arrival order.
  - RoPE's rotate_half is done with partition-shifted DVE multiplies
    (reading acc straight from PSUM via a bf16 copy) instead of a
    128x128 permutation matmul: -16k PE cycles.
  - softmax denominator: full pairwise DVE tree to one tile, then one
    short ones-matmul per half (issued inside the P item so the PE never
    waits on the exp->tree chain): -25k PE cycles vs v1.
  - phase 2 is a single software-pipelined worklist: scores(i) issue two
    heads ahead of PV(i) so the ScalarE exp backlog never stalls the PE;
    the 4th x-chunk's v-projection and the previous stripe's output
    projection pieces fill the remaining gaps.
  - output is written bf16 (host upcasts + sums partials): halves the
    out-DMA so the tail oproj isn't DMA-limited.
"""

import os
import numpy as np
import ml_dtypes

S, B, D = 2048, 2, 2048
N_HEADS, DQK = 16, 128
ROPE_THETA = 500000.0
N_CORES = 8
CORES_PER_BATCH = 4
NH_LOC = N_HEADS // CORES_PER_BATCH  # 4 heads per core

LAST_RESULT = None  # BassKernelResults of the most recent run (for test.py)

_NC_CACHE = {}


def _build_nc(s, dmodel, nh_loc, d=DQK, sc=512):
    import concourse.tile as tile
    from concourse import bacc, mybir

    bf16 = mybir.dt.bfloat16
    f32 = mybir.dt.float32
    nk = dmodel // 128      # contraction tiles for the projections
    ns = s // 128           # sequence tiles (key side)
    nch = s // sc           # sequence chunks (query side / moving dim)
    hd = nh_loc * d         # local head-dim total (512)
    ne = dmodel // 128      # output-embedding tiles
    sw = 2 * sc             # query stripe width (1024)
    nst = s // sw           # stripes (2)
    h2 = d // 2             # rotate_half split (64)

    nc = bacc.Bacc("TRN2", target_bir_lowering=False, debug=False)
    # inputs are pre-swizzled on the host to partition-major layouts so DMA
    # descriptors are 4-16KB contiguous runs instead of 1KB
    xTq = nc.dram_tensor("xTq", [nch, 128, nk, sc], bf16, kind="ExternalInput")
    wq = nc.dram_tensor("wq", [128, nk, hd], bf16, kind="ExternalInput")
    wk = nc.dram_tensor("wk", [128, nk, hd], bf16, kind="ExternalInput")
    wv = nc.dram_tensor("wv", [128, nk, hd], bf16, kind="ExternalInput")
    wo = nc.dram_tensor("wo", [128, hd // 128, dmodel], bf16,
                        kind="ExternalInput")
    cosT = nc.dram_tensor("cosT", [d, s], bf16, kind="ExternalInput")
    # sshT = sin.T with the BOTTOM 64 rows negated; the multiply for output
    # rows [0:64) reads ssh rows [64:128) (= -sin, base-partition-aligned
    # with raw[64:128)) and vice versa: rot(q)[p]*sin[p] = raw[p^64]*ssh[p^64]
    sshT = nc.dram_tensor("sshT", [d, s], bf16, kind="ExternalInput")
    maskb = nc.dram_tensor("maskb", [128, ns], f32, kind="ExternalInput")
    outT = nc.dram_tensor("outT", [dmodel, s], bf16, kind="ExternalOutput")

    outT_r = outT.rearrange("(e p) s -> p e s", p=128)

    with tile.TileContext(nc) as tc:
        with tc.tile_pool(name="const", bufs=1) as constp, \
             tc.tile_pool(name="store", bufs=1) as storep, \
             tc.tile_pool(name="psA", bufs=2, space="PSUM") as psA:
            qT_sb = storep.tile([128, nh_loc, s], bf16)
            kT_sb = storep.tile([128, nh_loc, s], bf16)
            v_sb = storep.tile([128, ns, hd], bf16)
            attn_sb = storep.tile([128, nh_loc, s], bf16)
            xc3 = storep.tile([128, nk, sc], bf16)  # chunk-3 x, used in ph2

            # ---- phase 1: q,k projection + rope; v for chunks 0-2 --------
            with tc.tile_pool(name="wqk", bufs=1) as wp, \
                 tc.tile_pool(name="tab", bufs=1) as tabp, \
                 tc.tile_pool(name="xch", bufs=2) as xp, \
                 tc.tile_pool(name="rope", bufs=2) as rp, \
                 tc.tile_pool(name="ps1", bufs=6, space="PSUM") as ps1:
                # per-k-tile DMAs in exactly the order the k-outer
                # accumulation consumes them
                wq_sb = wp.tile([128, nk, hd], bf16, tag="wq")
                wk_sb = wp.tile([128, nk, hd], bf16, tag="wk")
                xc0 = xp.tile([128, nk, sc], bf16, tag="xc")
                for k in range(0, nk, 2):
                    ks = slice(k, k + 2)
                    nc.sync.dma_start(wq_sb[:, ks, :], wq[:, ks, :])
                    nc.sync.dma_start(xc0[:, ks, :], xTq[0, :, ks, :])
                for k in range(0, nk, 4):
                    ks = slice(k, k + 4)
                    nc.sync.dma_start(wk_sb[:, ks, :], wk[:, ks, :])
                cos_sb = tabp.tile([128, s], bf16)
                nc.sync.dma_start(cos_sb[:], cosT[:])
                ssh_sb = tabp.tile([128, s], bf16)
                nc.sync.dma_start(ssh_sb[:], sshT[:])
                maskb_sb = constp.tile([128, ns], f32)
                nc.sync.dma_start(maskb_sb[:], maskb[:])
                wv_sb = constp.tile([128, nk, hd], bf16)
                gw = 4
                for g in range(0, nk, gw):
                    gs = slice(g, g + gw)
                    nc.sync.dma_start(wv_sb[:, gs, :], wv[:, gs, :])
                # chunk-1 x before wo: chunk 1's first matmul needs it ~45us
                # in, wo isn't needed until ~150us
                xcs = {0: xc0}
                xcs[1] = xp.tile([128, nk, sc], bf16, tag="xc", name="xc1")
                nc.sync.dma_start(xcs[1][:], xTq[1])
                wo_sb = constp.tile([128, nh_loc, dmodel], bf16)
                nc.sync.dma_start(wo_sb[:], wo[:])

                # warmup: keep the PE busy while the first DMAs land so the
                # HAM clock gate opens early.  The operands come from a memset
                # (no DMA dependency -> starts at ~0.5us); 512-row matmuls
                # keep the PE duty cycle high (the paired LDWEIGHTS is only
                # 128 rows), round-robining 4 PSUM slots to avoid WAW stalls.
                # ones_sb doubles as the denominator's summing stationary.
                ones_sb = constp.tile([128, sc], bf16)
                nc.vector.memset(ones_sb[:], 1.0)
                warms = []
                for _ in range(4):
                    warm = ps1.tile([128, sc], f32, tag="acc", name="warm")
                    warms.append(warm)
                for w in range(12):
                    warm = warms[w % 4]
                    nc.tensor.matmul(warm[:], ones_sb[:, :128], ones_sb[:],
                                     start=True, stop=True)

                def rope(acc, dstT, h, csl):
                    raw = rp.tile([128, sc], bf16, tag="raw")
                    nc.scalar.copy(raw[:], acc[:])
                    t1 = rp.tile([128, sc], bf16, tag="t1")
                    nc.vector.tensor_mul(t1[:], raw[:], cos_sb[:, csl])
                    t2 = rp.tile([128, sc], bf16, tag="t2")
                    # both SBUF inputs of a TensorTensor must share a base
                    # partition, so each multiply uses the ssh rows aligned
                    # with its raw slice (sin rows repeat: sin[m]==sin[m+64])
                    nc.vector.tensor_mul(t2[:h2, :], raw[h2:, :],
                                         ssh_sb[h2:, csl])
                    nc.vector.tensor_mul(t2[h2:, :], raw[:h2, :],
                                         ssh_sb[:h2, csl])
                    nc.vector.tensor_add(dstT[:, h, csl], t1[:], t2[:])

                for ch in range(nch):
                    csl = slice(ch * sc, (ch + 1) * sc)
                    # prefetch next chunk's x (chunks 0,1 pre-issued above)
                    nxt = ch + 1
                    if nxt < nch and nxt not in xcs:
                        if nxt == nch - 1:
                            xcs[nxt] = xc3
                        else:
                            xcs[nxt] = xp.tile([128, nk, sc], bf16, tag="xc",
                                               name="xcn")
                        nc.sync.dma_start(xcs[nxt][:], xTq[nxt])
                    xc = xcs[ch]
                    # q,k accumulation, k-tile-outer.  Last chunk does k
                    # heads first so kT (needed by the first scores) clears
                    # the rope pipeline early.
                    tlist = (("q", wq_sb, qT_sb), ("k", wk_sb, kT_sb))
                    if ch == nch - 1:
                        tlist = tlist[::-1]
                    for t, w_sb, dstT in tlist:
                        accs = []
                        for h in range(nh_loc):
                            acc = ps1.tile([128, sc], f32, tag="acc")
                            accs.append((acc, h))
                        for k in range(nk):
                            for acc, h in accs:
                                hsl = slice(h * d, (h + 1) * d)
                                nc.tensor.matmul(
                                    acc[:], w_sb[:, k, hsl], xc[:, k, :],
                                    start=(k == 0), stop=(k == nk - 1),
                                )
                        for acc, h in accs:
                            rope(acc, dstT, h, csl)
                    # v for chunks 0-2 (chunk 3 is deferred into phase 2)
                    if ch < nch - 1:
                        for stl in range(sc // 128):
                            st = ch * (sc // 128) + stl
                            ssl = slice(stl * 128, (stl + 1) * 128)
                            accv = ps1.tile([128, hd], f32, tag="acc")
                            for k in range(nk):
                                nc.tensor.matmul(
                                    accv[:], xc[:, k, ssl], wv_sb[:, k, :],
                                    start=(k == 0), stop=(k == nk - 1),
                                )
                            nc.scalar.copy(v_sb[:, st, :], accv[:])

            # ---- phase 2: attention + output projection ------------------
            with tc.tile_pool(name="expp", bufs=2) as expp, \
                 tc.tile_pool(name="tree", bufs=2) as treep, \
                 tc.tile_pool(name="invp", bufs=1) as invp, \
                 tc.tile_pool(name="outp", bufs=4) as outp, \
                 tc.tile_pool(name="ps2s", bufs=2, space="PSUM") as ps2s, \
                 tc.tile_pool(name="ps2p", bufs=2, space="PSUM") as ps2p:

                exs = {}    # i -> exp tile
                roots = {}  # i -> denominator tree root (SBUF bf16)

                def item_S(i):
                    ic, h = divmod(i, nh_loc)
                    ex = expp.tile([128, ns, sw], bf16, tag="exp")
                    exs[i] = ex
                    for jt in range(ns):
                        # weave an oproj unit in every 4 j-tiles: when the
                        # exp backlog paces the scores, this keeps PE busy
                        if jt in (4, 8, 12) and ofill:
                            o_unit()
                        jsl = slice(jt * 128, (jt + 1) * 128)
                        sc_ps = ps2s.tile([128, sw], f32, tag="scores")
                        for half in range(2):
                            qsl = slice(ic * sw + half * sc,
                                        ic * sw + (half + 1) * sc)
                            nc.tensor.matmul(
                                sc_ps[:, half * sc:(half + 1) * sc],
                                kT_sb[:, h, jsl], qT_sb[:, h, qsl],
                                start=True, stop=True)
                        nc.scalar.activation(
                            ex[:, jt, :], sc_ps[:],
                            mybir.ActivationFunctionType.Exp,
                            bias=maskb_sb[:, jt:jt + 1], scale=1.0,
                        )
                    # denominator part 1: running j-tile sum into one tile
                    # (DVE, paced by the exps as they land)
                    u = treep.tile([128, sw], bf16, tag="tree")
                    nc.vector.tensor_add(u[:], ex[:, 0, :], ex[:, 1, :])
                    for a in range(2, ns):
                        nc.vector.tensor_add(u[:], u[:], ex[:, a, :])
                    roots[i] = u

                def item_P(i):
                    ic, h = divmod(i, nh_loc)
                    hsl = slice(h * d, (h + 1) * d)
                    ex = exs.pop(i)
                    root = roots.pop(i)
                    # denominator part 2: sum the 128 j-partitions with one
                    # short ones-matmul per half.  Issued here (not in S)
                    # so the PE doesn't sit on the exp->tree dependency.
                    inv = invp.tile([128, sw], f32, tag="inv")
                    rps = []
                    for half in range(2):
                        fsl = slice(half * sc, (half + 1) * sc)
                        r_ps = psA.tile([128, sc], f32, tag="oproj",
                                         name="r_ps")
                        nc.tensor.matmul(r_ps[:], ones_sb[:, :128], root[:, fsl],
                                         start=True, stop=True)
                        rps.append((r_ps, fsl))
                    for r_ps, fsl in rps:
                        nc.vector.reciprocal_approx_fast(inv[:, fsl], r_ps[:])
                    for half in range(2):
                        fsl = slice(half * sc, (half + 1) * sc)
                        pv_ps = ps2p.tile([128, sc], f32, tag="pv")
                        for jt in range(ns):
                            nc.tensor.matmul(
                                pv_ps[:], v_sb[:, jt, hsl], ex[:, jt, fsl],
                                start=(jt == 0), stop=(jt == ns - 1),
                            )
                        asl = slice(ic * sw + half * sc,
                                    ic * sw + (half + 1) * sc)
                        nc.vector.tensor_mul(attn_sb[:, h, asl],
                                             pv_ps[:], inv[:, fsl])

                def item_V(st):
                    stl = st % (sc // 128)
                    ssl = slice(stl * 128, (stl + 1) * 128)
                    xcv = xc3
                    accv = psA.tile([128, hd], f32, tag="oproj")
                    for k in range(nk):
                        nc.tensor.matmul(
                            accv[:], xcv[:, k, ssl], wv_sb[:, k, :],
                            start=(k == 0), stop=(k == nk - 1),
                        )
                    nc.scalar.copy(v_sb[:, st, :], accv[:])

                # output projection, issued as (et, half) units that fill
                # PE slack between (and inside) the S/P items
                ofill = []
                ocnt = [0]

                def o_unit():
                    ic, et, half = ofill.pop(0)
                    esl = slice(et * 128, (et + 1) * 128)
                    osl = slice(ic * sw + half * sc,
                                ic * sw + (half + 1) * sc)
                    op_ps = psA.tile([128, sc], f32, tag="oproj")
                    for ht in range(nh_loc):
                        nc.tensor.matmul(
                            op_ps[:], wo_sb[:, ht, esl],
                            attn_sb[:, ht, osl],
                            start=(ht == 0), stop=(ht == nh_loc - 1),
                        )
                    ot = outp.tile([128, sc], bf16, tag="ot")
                    # stripe-0 units run while ScalarE is saturated with exp:
                    # drain on DVE only.  Tail units alternate DVE/ScalarE.
                    ocnt[0] += 1
                    if ic == 0 or ocnt[0] % 2 == 0:
                        nc.vector.tensor_copy(ot[:], op_ps[:])
                    else:
                        nc.scalar.copy(ot[:], op_ps[:])
                    nc.sync.dma_start(outT_r[:, et, osl], ot[:])

                v3 = (nch - 1) * (sc // 128)
                # S0 leads (kT clears the rope pipe ~4.5us in; scores jt0
                # waits briefly) so the ScalarE exp pipeline fills ASAP; the
                # scalar-free V items fill the exp-paced region instead of
                # the start, where ScalarE is idle anyway.
                sched = [("V", v3), ("S", 0), ("V", v3 + 1),
                         ("S", 1), ("V", v3 + 2), ("V", v3 + 3),
                         ("P", 0), ("S", 2), ("P", 1),
                         ("S", 3), ("P", 2),
                         ("S", 4), ("P", 3), ("U", 4), ("S", 5), ("U", 2),
                         ("P", 4), ("U", 4), ("S", 6), ("U", 2), ("P", 5),
                         ("U", 4), ("S", 7), ("U", 2), ("P", 6), ("U", 5),
                         ("P", 7)]
                assert nst == 2 and nh_loc == 4
                for kind, arg in sched:
                    if kind == "S":
                        item_S(arg)
                    elif kind == "P":
                        item_P(arg)
                        i = arg
                        if i % nh_loc == nh_loc - 1:
                            ic = i // nh_loc
                            ofill.extend((ic, et, half) for et in range(ne)
                                         for half in range(2))
                    elif kind == "V":
                        item_V(arg)
                    else:
                        for _ in range(min(arg, len(ofill))):
                            o_unit()
                while ofill:
                    o_unit()

    nc.compile()
    return nc


def _get_nc(s=S, dmodel=D, nh_loc=NH_LOC):
    key = (s, dmodel, nh_loc)
    if key not in _NC_CACHE:
        _NC_CACHE[key] = _build_nc(s, dmodel, nh_loc)
    return _NC_CACHE[key]


def _rope_tables(s, d, dtype=np.float32):
    inv_freq = 1.0 / (ROPE_THETA ** (np.arange(0, d, 2, dtype=np.float64) / d))
    pos = np.arange(s, dtype=np.float64)
    freqs = pos[:, None] * inv_freq[None, :]            # [s, d/2]
    emb = np.concatenate([freqs, freqs], axis=-1)       # [s, d]
    return np.cos(emb).astype(dtype), np.sin(emb).astype(dtype)


def make_in_maps(hidden_states, sequence_mask, Wqkv, Wo,
                 s=S, b=B, dmodel=D, nh_tot=N_HEADS, nh_loc=NH_LOC, d=DQK):
    bf = ml_dtypes.bfloat16
    cos, sin = _rope_tables(s, d)
    cosT = np.ascontiguousarray(cos.T).astype(bf)       # [d, s]
    sinT = np.ascontiguousarray(sin.T)                  # [d, s] f32
    ssh = sinT.copy()
    ssh[d // 2:] = -ssh[d // 2:]
    sshT = ssh.astype(bf)
    scale = 1.0 / np.sqrt(np.float32(d))

    in_maps = []
    cores_per_batch = N_CORES // b
    for c in range(N_CORES):
        bi = c // cores_per_batch
        g = c % cores_per_batch
        h0 = g * nh_loc
        hsl = slice(h0 * d, (h0 + nh_loc) * d)
        nk, sc, nch = dmodel // 128, 512, s // 512
        xb = hidden_states[:, bi, :]                    # [s, dmodel]
        # [nch, 128, nk, sc]: xTq[ch, p, k, j] = x[ch*sc+j, k*128+p]
        xTq = np.ascontiguousarray(
            xb.T.reshape(nk, 128, nch, sc).transpose(2, 1, 0, 3)).astype(bf)
        hd = nh_loc * d

        def swz_w(w):   # [dmodel, hd] -> [128, nk, hd]
            return np.ascontiguousarray(
                w.reshape(nk, 128, hd).transpose(1, 0, 2)).astype(bf)

        wq = swz_w(Wqkv[:, 0 * nh_tot * d:1 * nh_tot * d][:, hsl] * scale)
        wk = swz_w(Wqkv[:, 1 * nh_tot * d:2 * nh_tot * d][:, hsl])
        wv = swz_w(Wqkv[:, 2 * nh_tot * d:3 * nh_tot * d][:, hsl])
        # [128, nh_loc, dmodel]: wo[p, h, e] = Wo[h*128+p, e]
        wo = np.ascontiguousarray(
            Wo[hsl, :].reshape(nh_loc, 128, dmodel).transpose(1, 0, 2)
        ).astype(bf)
        bias = np.where(sequence_mask[bi] == 0, -1e30, 0.0).astype(np.float32)
        maskbT = np.ascontiguousarray(bias.reshape(s // 128, 128).T)  # [128, ns]
        in_maps.append({
            "xTq": xTq, "wq": wq, "wk": wk, "wv": wv, "wo": wo,
            "cosT": cosT, "sshT": sshT, "maskb": maskbT,
        })
    return in_maps


def kernel(hidden_states, sequence_mask, Wqkv, Wo):
    global LAST_RESULT
    from concourse.bass_utils import run_bass_kernel_spmd

    hidden_states = np.asarray(hidden_states)
    sequence_mask = np.asarray(sequence_mask)
    Wqkv = np.asarray(Wqkv)
    Wo = np.asarray(Wo)

    nc = _get_nc()
    in_maps = make_in_maps(hidden_states, sequence_mask, Wqkv, Wo)
    res = run_bass_kernel_spmd(
        nc, in_maps, list(range(N_CORES)),
        trace=bool(int(os.environ.get("KERNEL_TRACE", "0"))),
    )
    LAST_RESULT = res

    out = np.empty((S, B, D), dtype=np.float32)
    cores_per_batch = N_CORES // B
    for bi in range(B):
        acc = None
        for g in range(cores_per_batch):
            part = res.results[bi * cores_per_batch + g]["outT"]  # [D, S] bf16
            part = np.asarray(part, dtype=np.float32)
            acc = part if acc is None else acc + part
        out[:, bi, :] = acc.T
    return out


# revision 20
# speedup vs baseline: 1.0175x; 1.0004x over previous
"""Trainium2 Bass kernel for 16-head RoPE self-attention (S=2048, B=2, D=2048).

Sharding: 8 cores = 2 batches x 4 head-groups (4 heads each). Each core
computes qkv projection for its batch/heads, full attention over its 4
heads, and a partial output projection (its 4-head slice of Wo rows).
Host sums the 4 partial outputs per batch. No cross-core collectives.

The kernel is PE-bound (~92% tensor busy), so every change targets PE
busy-cycles or PE idle gaps:
  - warmup block: 16 dummy 512-row matmuls on a memset tile at kernel
    start keep the PE HAM activity window busy during the initial
    weight/x DMA, so the clock gate opens at ~11us instead of ~25us.
  - qkv projection runs k-tile-outer x head-inner so the first chunk's
    accumulation consumes weight/x tiles in DMA arrival order.
  - RoPE's rotate_half is done with partition-shifted DVE multiplies
    (reading acc straight from PSUM via a bf16 copy) instead of a
    128x128 permutation matmul: -16k PE cycles.
  - softmax denominator: full pairwise DVE tree to one tile, then one
    short ones-matmul per half (issued inside the P item so the PE never
    waits on the exp->tree chain): -25k PE cycles vs v1.
  - phase 2 is a single software-pipelined worklist: scores(i) issue two
    heads ahead of PV(i) so the ScalarE exp backlog never stalls the PE;
    the 4th x-chunk's v-projection and the previous stripe's output
    projection pieces fill the remaining gaps.
  - output is written bf16 (host upcasts + sums partials): halves the
    out-DMA so the tail oproj isn't DMA-limited.
"""

import os
import numpy as np
import ml_dtypes

S, B, D = 2048, 2, 2048
N_HEADS, DQK = 16, 128
ROPE_THETA = 500000.0
N_CORES = 8
CORES_PER_BATCH = 4
NH_LOC = N_HEADS // CORES_PER_BATCH  # 4 heads per core

LAST_RESULT = None  # BassKernelResults of the most recent run (for test.py)

_NC_CACHE = {}


def _build_nc(s, dmodel, nh_loc, d=DQK, sc=512):
    import concourse.tile as tile
    from concourse import bacc, mybir

    bf16 = mybir.dt.bfloat16
    f32 = mybir.dt.float32
    nk = dmodel // 128      # contraction tiles for the projections
    ns = s // 128           # sequence tiles (key side)
    nch = s // sc           # sequence chunks (query side / moving dim)
    hd = nh_loc * d         # local head-dim total (512)
    ne = dmodel // 128      # output-embedding tiles
    sw = 2 * sc             # query stripe width (1024)
    nst = s // sw           # stripes (2)
    h2 = d // 2             # rotate_half split (64)

    nc = bacc.Bacc("TRN2", target_bir_lowering=False, debug=False)
    # inputs are pre-swizzled on the host to partition-major layouts so DMA
    # descriptors are 4-16KB contiguous runs instead of 1KB
    xTq = nc.dram_tensor("xTq", [nch, 128, nk, sc], bf16, kind="ExternalInput")
    wq = nc.dram_tensor("wq", [128, nk, hd], bf16, kind="ExternalInput")
    wk = nc.dram_tensor("wk", [128, nk, hd], bf16, kind="ExternalInput")
    wv = nc.dram_tensor("wv", [128, nk, hd], bf16, kind="ExternalInput")
    wo = nc.dram_tensor("wo", [128, hd // 128, dmodel], bf16,
                        kind="ExternalInput")
    cosT = nc.dram_tensor("cosT", [d, s], bf16, kind="ExternalInput")
    # sshT = sin.T with the BOTTOM 64 rows negated; the multiply for output
    # rows [0:64) reads ssh rows [64:128) (= -sin, base-partition-aligned
    # with raw[64:128)) and vice versa: rot(q)[p]*sin[p] = raw[p^64]*ssh[p^64]
    sshT = nc.dram_tensor("sshT", [d, s], bf16, kind="ExternalInput")
    maskb = nc.dram_tensor("maskb", [128, ns], f32, kind="ExternalInput")
    outT = nc.dram_tensor("outT", [dmodel, s], bf16, kind="ExternalOutput")

    outT_r = outT.rearrange("(e p) s -> p e s", p=128)

    with tile.TileContext(nc) as tc:
        with tc.tile_pool(name="const", bufs=1) as constp, \
             tc.tile_pool(name="store", bufs=1) as storep, \
             tc.tile_pool(name="psA", bufs=2, space="PSUM") as psA:
            qT_sb = storep.tile([128, nh_loc, s], bf16)
            kT_sb = storep.tile([128, nh_loc, s], bf16)
            v_sb = storep.tile([128, ns, hd], bf16)
            attn_sb = storep.tile([128, nh_loc, s], bf16)
            xc3 = storep.tile([128, nk, sc], bf16)  # chunk-3 x, used in ph2

            # ---- phase 1: q,k projection + rope; v for chunks 0-2 --------
            with tc.tile_pool(name="wqk", bufs=1) as wp, \
                 tc.tile_pool(name="tab", bufs=1) as tabp, \
                 tc.tile_pool(name="xch", bufs=2) as xp, \
                 tc.tile_pool(name="rope", bufs=2) as rp, \
                 tc.tile_pool(name="ps1", bufs=6, space="PSUM") as ps1:
                # per-k-tile DMAs in exactly the order the k-outer
                # accumulation consumes them
                wq_sb = wp.tile([128, nk, hd], bf16, tag="wq")
                wk_sb = wp.tile([128, nk, hd], bf16, tag="wk")
                xc0 = xp.tile([128, nk, sc], bf16, tag="xc")
                for k in range(0, nk, 2):
                    ks = slice(k, k + 2)
                    nc.sync.dma_start(wq_sb[:, ks, :], wq[:, ks, :])
                    nc.sync.dma_start(xc0[:, ks, :], xTq[0, :, ks, :])
                for k in range(0, nk, 4):
                    ks = slice(k, k + 4)
                    nc.sync.dma_start(wk_sb[:, ks, :], wk[:, ks, :])
                cos_sb = tabp.tile([128, s], bf16)
                nc.sync.dma_start(cos_sb[:], cosT[:])
                ssh_sb = tabp.tile([128, s], bf16)
                nc.sync.dma_start(ssh_sb[:], sshT[:])
                maskb_sb = constp.tile([128, ns], f32)
                nc.sync.dma_start(maskb_sb[:], maskb[:])
                wv_sb = constp.tile([128, nk, hd], bf16)
                gw = 4
                for g in range(0, nk, gw):
                    gs = slice(g, g + gw)
                    nc.sync.dma_start(wv_sb[:, gs, :], wv[:, gs, :])
                # chunk-1 x before wo: chunk 1's first matmul needs it ~45us
                # in, wo isn't needed until ~150us
                xcs = {0: xc0}
                xcs[1] = xp.tile([128, nk, sc], bf16, tag="xc", name="xc1")
                nc.sync.dma_start(xcs[1][:], xTq[1])
                wo_sb = constp.tile([128, nh_loc, dmodel], bf16)
                nc.sync.dma_start(wo_sb[:], wo[:])

                # warmup: keep the PE busy while the first DMAs land so the
                # HAM clock gate opens early.  The operands come from a memset
                # (no DMA dependency -> starts at ~0.5us); 512-row matmuls
                # keep the PE duty cycle high (the paired LDWEIGHTS is only
                # 128 rows), round-robining 4 PSUM slots to avoid WAW stalls.
                # ones_sb doubles as the denominator's summing stationary.
                # the warmup matmuls read ones_sb BEFORE the memset: their
                # output is never read, so uninitialized data is fine, and
                # skipping the memset wait lets the PE start ~2.5us earlier.
                # The memset (WAR-ordered after the last warm matmul) then
                # fills the real all-ones operand for the denominator sums.
                ones_sb = constp.tile([128, sc], bf16)
                warms = []
                for _ in range(4):
                    warm = ps1.tile([128, sc], f32, tag="acc", name="warm")
                    warms.append(warm)
                for w in range(12):
                    warm = warms[w % 4]
                    nc.tensor.matmul(warm[:], ones_sb[:, :128], ones_sb[:],
                                     start=True, stop=True)
                nc.vector.memset(ones_sb[:], 1.0)

                def rope(acc, dstT, h, csl):
                    raw = rp.tile([128, sc], bf16, tag="raw")
                    nc.scalar.copy(raw[:], acc[:])
                    t1 = rp.tile([128, sc], bf16, tag="t1")
                    nc.vector.tensor_mul(t1[:], raw[:], cos_sb[:, csl])
                    t2 = rp.tile([128, sc], bf16, tag="t2")
                    # both SBUF inputs of a TensorTensor must share a base
                    # partition, so each multiply uses the ssh rows aligned
                    # with its raw slice (sin rows repeat: sin[m]==sin[m+64])
                    nc.vector.tensor_mul(t2[:h2, :], raw[h2:, :],
                                         ssh_sb[h2:, csl])
                    nc.vector.tensor_mul(t2[h2:, :], raw[:h2, :],
                                         ssh_sb[:h2, csl])
                    nc.vector.tensor_add(dstT[:, h, csl], t1[:], t2[:])

                for ch in range(nch):
                    csl = slice(ch * sc, (ch + 1) * sc)
                    # prefetch next chunk's x (chunks 0,1 pre-issued above)
                    nxt = ch + 1
                    if nxt < nch and nxt not in xcs:
                        if nxt == nch - 1:
                            xcs[nxt] = xc3
                        else:
                            xcs[nxt] = xp.tile([128, nk, sc], bf16, tag="xc",
                                               name="xcn")
                        nc.sync.dma_start(xcs[nxt][:], xTq[nxt])
                    xc = xcs[ch]
                    # q,k accumulation, k-tile-outer.  Last chunk does k
                    # heads first so kT (needed by the first scores) clears
                    # the rope pipeline early.
                    tlist = (("q", wq_sb, qT_sb), ("k", wk_sb, kT_sb))
                    if ch == nch - 1:
                        tlist = tlist[::-1]
                    for t, w_sb, dstT in tlist:
                        accs = []
                        for h in range(nh_loc):
                            acc = ps1.tile([128, sc], f32, tag="acc")
                            accs.append((acc, h))
                        for k in range(nk):
                            for acc, h in accs:
                                hsl = slice(h * d, (h + 1) * d)
                                nc.tensor.matmul(
                                    acc[:], w_sb[:, k, hsl], xc[:, k, :],
                                    start=(k == 0), stop=(k == nk - 1),
                                )
                        for acc, h in accs:
                            rope(acc, dstT, h, csl)
                    # v for chunks 0-2 (chunk 3 is deferred into phase 2)
                    if ch < nch - 1:
                        for stl in range(sc // 128):
                            st = ch * (sc // 128) + stl
                            ssl = slice(stl * 128, (stl + 1) * 128)
                            accv = ps1.tile([128, hd], f32, tag="acc")
                            for k in range(nk):
                                nc.tensor.matmul(
                                    accv[:], xc[:, k, ssl], wv_sb[:, k, :],
                                    start=(k == 0), stop=(k == nk - 1),
                                )
                            nc.scalar.copy(v_sb[:, st, :], accv[:])

            # ---- phase 2: attention + output projection ------------------
            with tc.tile_pool(name="expp", bufs=2) as expp, \
                 tc.tile_pool(name="tree", bufs=2) as treep, \
                 tc.tile_pool(name="invp", bufs=1) as invp, \
                 tc.tile_pool(name="outp", bufs=4) as outp, \
                 tc.tile_pool(name="ps2s", bufs=2, space="PSUM") as ps2s, \
                 tc.tile_pool(name="ps2p", bufs=2, space="PSUM") as ps2p:

                exs = {}    # i -> exp tile
                roots = {}  # i -> denominator tree root (SBUF bf16)

                def item_S(i):
                    ic, h = divmod(i, nh_loc)
                    ex = expp.tile([128, ns, sw], bf16, tag="exp")
                    exs[i] = ex
                    for jt in range(ns):
                        # weave an oproj unit in every 4 j-tiles: when the
                        # exp backlog paces the scores, this keeps PE busy
                        if jt in (4, 8, 12) and ofill:
                            o_unit()
                        jsl = slice(jt * 128, (jt + 1) * 128)
                        sc_ps = ps2s.tile([128, sw], f32, tag="scores")
                        for half in range(2):
                            qsl = slice(ic * sw + half * sc,
                                        ic * sw + (half + 1) * sc)
                            nc.tensor.matmul(
                                sc_ps[:, half * sc:(half + 1) * sc],
                                kT_sb[:, h, jsl], qT_sb[:, h, qsl],
                                start=True, stop=True)
                        nc.scalar.activation(
                            ex[:, jt, :], sc_ps[:],
                            mybir.ActivationFunctionType.Exp,
                            bias=maskb_sb[:, jt:jt + 1], scale=1.0,
                        )
                    # denominator part 1: running j-tile sum into one tile
                    # (DVE, paced by the exps as they land)
                    u = treep.tile([128, sw], bf16, tag="tree")
                    nc.vector.tensor_add(u[:], ex[:, 0, :], ex[:, 1, :])
                    for a in range(2, ns):
                        nc.vector.tensor_add(u[:], u[:], ex[:, a, :])
                    roots[i] = u

                def item_P(i):
                    ic, h = divmod(i, nh_loc)
                    hsl = slice(h * d, (h + 1) * d)
                    ex = exs.pop(i)
                    root = roots.pop(i)
                    # denominator part 2: sum the 128 j-partitions with one
                    # short ones-matmul per half.  Issued here (not in S)
                    # so the PE doesn't sit on the exp->tree dependency.
                    inv = invp.tile([128, sw], f32, tag="inv")
                    rps = []
                    for half in range(2):
                        fsl = slice(half * sc, (half + 1) * sc)
                        r_ps = psA.tile([128, sc], f32, tag="oproj",
                                         name="r_ps")
                        nc.tensor.matmul(r_ps[:], ones_sb[:, :128], root[:, fsl],
                                         start=True, stop=True)
                        rps.append((r_ps, fsl))
                    for r_ps, fsl in rps:
                        nc.vector.reciprocal_approx_fast(inv[:, fsl], r_ps[:])
                    for half in range(2):
                        fsl = slice(half * sc, (half + 1) * sc)
                        pv_ps = ps2p.tile([128, sc], f32, tag="pv")
                        for jt in range(ns):
                            nc.tensor.matmul(
                                pv_ps[:], v_sb[:, jt, hsl], ex[:, jt, fsl],
                                start=(jt == 0), stop=(jt == ns - 1),
                            )
                        asl = slice(ic * sw + half * sc,
                                    ic * sw + (half + 1) * sc)
                        nc.vector.tensor_mul(attn_sb[:, h, asl],
                                             pv_ps[:], inv[:, fsl])

                def item_V(st):
                    stl = st % (sc // 128)
                    ssl = slice(stl * 128, (stl + 1) * 128)
                    xcv = xc3
                    accv = psA.tile([128, hd], f32, tag="oproj")
                    for k in range(nk):
                        nc.tensor.matmul(
                            accv[:], xcv[:, k, ssl], wv_sb[:, k, :],
                            start=(k == 0), stop=(k == nk - 1),
                        )
                    nc.scalar.copy(v_sb[:, st, :], accv[:])

                # output projection, issued as (et, half) units that fill
                # PE slack between (and inside) the S/P items
                ofill = []
                ocnt = [0]

                def o_unit():
                    ic, et, half = ofill.pop(0)
                    esl = slice(et * 128, (et + 1) * 128)
                    osl = slice(ic * sw + half * sc,
                                ic * sw + (half + 1) * sc)
                    op_ps = psA.tile([128, sc], f32, tag="oproj")
                    for ht in range(nh_loc):
                        nc.tensor.matmul(
                            op_ps[:], wo_sb[:, ht, esl],
                            attn_sb[:, ht, osl],
                            start=(ht == 0), stop=(ht == nh_loc - 1),
                        )
                    ot = outp.tile([128, sc], bf16, tag="ot")
                    # stripe-0 units run while ScalarE is saturated with exp:
                    # drain on DVE only.  Tail units alternate DVE/ScalarE.
                    ocnt[0] += 1
                    if ic == 0 or ocnt[0] % 2 == 0:
                        nc.vector.tensor_copy(ot[:], op_ps[:])
                    else:
                        nc.scalar.copy(ot[:], op_ps[:])
                    nc.sync.dma_start(outT_r[:, et, osl], ot[:])

                v3 = (nch - 1) * (sc // 128)
                # S0 leads (kT clears the rope pipe ~4.5us in; scores jt0
                # waits briefly) so the ScalarE exp pipeline fills ASAP; the
                # scalar-free V items fill the exp-paced region instead of
                # the start, where ScalarE is idle anyway.
                sched = [("V", v3), ("S", 0), ("V", v3 + 1),
                         ("S", 1), ("V", v3 + 2), ("V", v3 + 3),
                         ("P", 0), ("S", 2), ("P", 1),
                         ("S", 3), ("P", 2),
                         ("S", 4), ("P", 3), ("U", 4), ("S", 5), ("U", 2),
                         ("P", 4), ("U", 4), ("S", 6), ("U", 2), ("P", 5),
                         ("U", 4), ("S", 7), ("U", 2), ("P", 6), ("U", 5),
                         ("P", 7)]
                assert nst == 2 and nh_loc == 4
                for kind, arg in sched:
                    if kind == "S":
                        item_S(arg)
                    elif kind == "P":
                        item_P(arg)
                        i = arg
                        if i % nh_loc == nh_loc - 1:
                            ic = i // nh_loc
                            ofill.extend((ic, et, half) for et in range(ne)
                                         for half in range(2))
                    elif kind == "V":
                        item_V(arg)
                    else:
                        for _ in range(min(arg, len(ofill))):
                            o_unit()
                while ofill:
                    o_unit()

    nc.compile()
    return nc


def _get_nc(s=S, dmodel=D, nh_loc=NH_LOC):
    key = (s, dmodel, nh_loc)
    if key not in _NC_CACHE:
        _NC_CACHE[key] = _build_nc(s, dmodel, nh_loc)
    return _NC_CACHE[key]


def _rope_tables(s, d, dtype=np.float32):
    inv_freq = 1.0 / (ROPE_THETA ** (np.arange(0, d, 2, dtype=np.float64) / d))
    pos = np.arange(s, dtype=np.float64)
    freqs = pos[:, None] * inv_freq[None, :]            # [s, d/2]
    emb = np.concatenate([freqs, freqs], axis=-1)       # [s, d]
    return np.cos(emb).astype(dtype), np.sin(emb).astype(dtype)


def make_in_maps(hidden_states, sequence_mask, Wqkv, Wo,
                 s=S, b=B, dmodel=D, nh_tot=N_HEADS, nh_loc=NH_LOC, d=DQK):
    bf = ml_dtypes.bfloat16
    cos, sin = _rope_tables(s, d)
    cosT = np.ascontiguousarray(cos.T).astype(bf)       # [d, s]
    sinT = np.ascontiguousarray(sin.T)                  # [d, s] f32
    ssh = sinT.copy()
    ssh[d // 2:] = -ssh[d // 2:]
    sshT = ssh.astype(bf)
    scale = 1.0 / np.sqrt(np.float32(d))

    in_maps = []
    cores_per_batch = N_CORES // b
    for c in range(N_CORES):
        bi = c // cores_per_batch
        g = c % cores_per_batch
        h0 = g * nh_loc
        hsl = slice(h0 * d, (h0 + nh_loc) * d)
        nk, sc, nch = dmodel // 128, 512, s // 512
        xb = hidden_states[:, bi, :]                    # [s, dmodel]
        # [nch, 128, nk, sc]: xTq[ch, p, k, j] = x[ch*sc+j, k*128+p]
        xTq = np.ascontiguousarray(
            xb.T.reshape(nk, 128, nch, sc).transpose(2, 1, 0, 3)).astype(bf)
        hd = nh_loc * d

        def swz_w(w):   # [dmodel, hd] -> [128, nk, hd]
            return np.ascontiguousarray(
                w.reshape(nk, 128, hd).transpose(1, 0, 2)).astype(bf)

        wq = swz_w(Wqkv[:, 0 * nh_tot * d:1 * nh_tot * d][:, hsl] * scale)
        wk = swz_w(Wqkv[:, 1 * nh_tot * d:2 * nh_tot * d][:, hsl])
        wv = swz_w(Wqkv[:, 2 * nh_tot * d:3 * nh_tot * d][:, hsl])
        # [128, nh_loc, dmodel]: wo[p, h, e] = Wo[h*128+p, e]
        wo = np.ascontiguousarray(
            Wo[hsl, :].reshape(nh_loc, 128, dmodel).transpose(1, 0, 2)
        ).astype(bf)
        bias = np.where(sequence_mask[bi] == 0, -1e30, 0.0).astype(np.float32)
        maskbT = np.ascontiguousarray(bias.reshape(s // 128, 128).T)  # [128, ns]
        in_maps.append({
            "xTq": xTq, "wq": wq, "wk": wk, "wv": wv, "wo": wo,
            "cosT": cosT, "sshT": sshT, "maskb": maskbT,
        })
    return in_maps


def kernel(hidden_states, sequence_mask, Wqkv, Wo):
    global LAST_RESULT
    from concourse.bass_utils import run_bass_kernel_spmd

    hidden_states = np.asarray(hidden_states)
    sequence_mask = np.asarray(sequence_mask)
    Wqkv = np.asarray(Wqkv)
    Wo = np.asarray(Wo)

    nc = _get_nc()
    in_maps = make_in_maps(hidden_states, sequence_mask, Wqkv, Wo)
    res = run_bass_kernel_spmd(
        nc, in_maps, list(range(N_CORES)),
        trace=bool(int(os.environ.get("KERNEL_TRACE", "0"))),
    )
    LAST_RESULT = res

    out = np.empty((S, B, D), dtype=np.float32)
    cores_per_batch = N_CORES // B
    for bi in range(B):
        acc = None
        for g in range(cores_per_batch):
            part = res.results[bi * cores_per_batch + g]["outT"]  # [D, S] bf16
            part = np.asarray(part, dtype=np.float32)
            acc = part if acc is None else acc + part
        out[:, bi, :] = acc.T
    return out


# revision 21
# speedup vs baseline: 1.0263x; 1.0087x over previous
"""Trainium2 Bass kernel for 16-head RoPE self-attention (S=2048, B=2, D=2048).

Sharding: 8 cores = 2 batches x 4 head-groups (4 heads each). Each core
computes qkv projection for its batch/heads, full attention over its 4
heads, and a partial output projection (its 4-head slice of Wo rows).
Host sums the 4 partial outputs per batch. No cross-core collectives.

The kernel is PE-bound (~92% tensor busy), so every change targets PE
busy-cycles or PE idle gaps:
  - warmup block: 16 dummy 512-row matmuls on a memset tile at kernel
    start keep the PE HAM activity window busy during the initial
    weight/x DMA, so the clock gate opens at ~11us instead of ~25us.
  - qkv projection runs k-tile-outer x head-inner so the first chunk's
    accumulation consumes weight/x tiles in DMA arrival order.
  - RoPE's rotate_half is done with partition-shifted DVE multiplies
    (reading acc straight from PSUM via a bf16 copy) instead of a
    128x128 permutation matmul: -16k PE cycles.
  - softmax denominator: full pairwise DVE tree to one tile, then one
    short ones-matmul per half (issued inside the P item so the PE never
    waits on the exp->tree chain): -25k PE cycles vs v1.
  - phase 2 is a single software-pipelined worklist: scores(i) issue two
    heads ahead of PV(i) so the ScalarE exp backlog never stalls the PE;
    the 4th x-chunk's v-projection and the previous stripe's output
    projection pieces fill the remaining gaps.
  - output is written bf16 (host upcasts + sums partials): halves the
    out-DMA so the tail oproj isn't DMA-limited.
"""

import os
import numpy as np
import ml_dtypes

S, B, D = 2048, 2, 2048
N_HEADS, DQK = 16, 128
ROPE_THETA = 500000.0
N_CORES = 8
CORES_PER_BATCH = 4
NH_LOC = N_HEADS // CORES_PER_BATCH  # 4 heads per core

LAST_RESULT = None  # BassKernelResults of the most recent run (for test.py)

_NC_CACHE = {}


def _build_nc(s, dmodel, nh_loc, d=DQK, sc=512):
    import concourse.tile as tile
    from concourse import bacc, mybir

    bf16 = mybir.dt.bfloat16
    f32 = mybir.dt.float32
    nk = dmodel // 128      # contraction tiles for the projections
    ns = s // 128           # sequence tiles (key side)
    nch = s // sc           # sequence chunks (query side / moving dim)
    hd = nh_loc * d         # local head-dim total (512)
    ne = dmodel // 128      # output-embedding tiles
    sw = 2 * sc             # query stripe width (1024)
    nst = s // sw           # stripes (2)
    h2 = d // 2             # rotate_half split (64)

    nc = bacc.Bacc("TRN2", target_bir_lowering=False, debug=False)
    # inputs are pre-swizzled on the host to partition-major layouts so DMA
    # descriptors are 4-16KB contiguous runs instead of 1KB
    xTq = nc.dram_tensor("xTq", [nch, 128, nk, sc], bf16, kind="ExternalInput")
    wq = nc.dram_tensor("wq", [128, nk, hd], bf16, kind="ExternalInput")
    wk = nc.dram_tensor("wk", [128, nk, hd], bf16, kind="ExternalInput")
    wv = nc.dram_tensor("wv", [128, nk, hd], bf16, kind="ExternalInput")
    wo = nc.dram_tensor("wo", [128, hd // 128, dmodel], bf16,
                        kind="ExternalInput")
    cosT = nc.dram_tensor("cosT", [d, s], bf16, kind="ExternalInput")
    # sshT = sin.T with the BOTTOM 64 rows negated; the multiply for output
    # rows [0:64) reads ssh rows [64:128) (= -sin, base-partition-aligned
    # with raw[64:128)) and vice versa: rot(q)[p]*sin[p] = raw[p^64]*ssh[p^64]
    sshT = nc.dram_tensor("sshT", [d, s], bf16, kind="ExternalInput")
    maskb = nc.dram_tensor("maskb", [128, ns], f32, kind="ExternalInput")
    outT = nc.dram_tensor("outT", [dmodel, s], bf16, kind="ExternalOutput")

    outT_r = outT.rearrange("(e p) s -> p e s", p=128)

    with tile.TileContext(nc) as tc:
        with tc.tile_pool(name="const", bufs=1) as constp, \
             tc.tile_pool(name="store", bufs=1) as storep, \
             tc.tile_pool(name="psA", bufs=2, space="PSUM") as psA:
            qT_sb = storep.tile([128, nh_loc, s], bf16)
            kT_sb = storep.tile([128, nh_loc, s], bf16)
            v_sb = storep.tile([128, ns, hd], bf16)
            attn_sb = storep.tile([128, nh_loc, s], bf16)
            xc3 = storep.tile([128, nk, sc], bf16)  # chunk-3 x, used in ph2

            # ---- phase 1: q,k projection + rope; v for chunks 0-2 --------
            with tc.tile_pool(name="wqk", bufs=1) as wp, \
                 tc.tile_pool(name="tab", bufs=1) as tabp, \
                 tc.tile_pool(name="xch", bufs=2) as xp, \
                 tc.tile_pool(name="rope", bufs=2) as rp, \
                 tc.tile_pool(name="ps1", bufs=6, space="PSUM") as ps1:
                # per-k-tile DMAs in exactly the order the k-outer
                # accumulation consumes them
                wq_sb = wp.tile([128, nk, hd], bf16, tag="wq")
                wk_sb = wp.tile([128, nk, hd], bf16, tag="wk")
                xc0 = xp.tile([128, nk, sc], bf16, tag="xc")
                for k in range(0, nk, 2):
                    ks = slice(k, k + 2)
                    nc.sync.dma_start(wq_sb[:, ks, :], wq[:, ks, :])
                    nc.sync.dma_start(xc0[:, ks, :], xTq[0, :, ks, :])
                for k in range(0, nk, 4):
                    ks = slice(k, k + 4)
                    nc.sync.dma_start(wk_sb[:, ks, :], wk[:, ks, :])
                cos_sb = tabp.tile([128, s], bf16)
                nc.sync.dma_start(cos_sb[:], cosT[:])
                ssh_sb = tabp.tile([128, s], bf16)
                nc.sync.dma_start(ssh_sb[:], sshT[:])
                maskb_sb = constp.tile([128, ns], f32)
                nc.sync.dma_start(maskb_sb[:], maskb[:])
                wv_sb = constp.tile([128, nk, hd], bf16)
                gw = 4
                for g in range(0, nk, gw):
                    gs = slice(g, g + gw)
                    nc.sync.dma_start(wv_sb[:, gs, :], wv[:, gs, :])
                # chunk-1 x before wo: chunk 1's first matmul needs it ~45us
                # in, wo isn't needed until ~150us
                xcs = {0: xc0}
                xcs[1] = xp.tile([128, nk, sc], bf16, tag="xc", name="xc1")
                nc.sync.dma_start(xcs[1][:], xTq[1])
                wo_sb = constp.tile([128, nh_loc, dmodel], bf16)
                nc.sync.dma_start(wo_sb[:], wo[:])

                # warmup: keep the PE busy while the first DMAs land so the
                # HAM clock gate opens early.  The operands come from a memset
                # (no DMA dependency -> starts at ~0.5us); 512-row matmuls
                # keep the PE duty cycle high (the paired LDWEIGHTS is only
                # 128 rows), round-robining 4 PSUM slots to avoid WAW stalls.
                # ones_sb doubles as the denominator's summing stationary.
                # the warmup matmuls read ones_sb BEFORE the memset: their
                # output is never read, so uninitialized data is fine, and
                # skipping the memset wait lets the PE start ~2.5us earlier.
                # The memset (WAR-ordered after the last warm matmul) then
                # fills the real all-ones operand for the denominator sums.
                ones_sb = constp.tile([128, sc], bf16)
                warms = []
                for _ in range(4):
                    warm = ps1.tile([128, sc], f32, tag="acc", name="warm")
                    warms.append(warm)
                for w in range(12):
                    warm = warms[w % 4]
                    nc.tensor.matmul(warm[:], ones_sb[:, :128], ones_sb[:],
                                     start=True, stop=True)
                nc.vector.memset(ones_sb[:], 1.0)

                def rope(acc, dstT, h, csl):
                    raw = rp.tile([128, sc], bf16, tag="raw")
                    nc.scalar.copy(raw[:], acc[:])
                    t1 = rp.tile([128, sc], bf16, tag="t1")
                    nc.vector.tensor_mul(t1[:], raw[:], cos_sb[:, csl])
                    t2 = rp.tile([128, sc], bf16, tag="t2")
                    # both SBUF inputs of a TensorTensor must share a base
                    # partition, so each multiply uses the ssh rows aligned
                    # with its raw slice (sin rows repeat: sin[m]==sin[m+64])
                    nc.vector.tensor_mul(t2[:h2, :], raw[h2:, :],
                                         ssh_sb[h2:, csl])
                    nc.vector.tensor_mul(t2[h2:, :], raw[:h2, :],
                                         ssh_sb[:h2, csl])
                    nc.vector.tensor_add(dstT[:, h, csl], t1[:], t2[:])

                for ch in range(nch):
                    csl = slice(ch * sc, (ch + 1) * sc)
                    # prefetch next chunk's x (chunks 0,1 pre-issued above)
                    nxt = ch + 1
                    if nxt < nch and nxt not in xcs:
                        if nxt == nch - 1:
                            xcs[nxt] = xc3
                        else:
                            xcs[nxt] = xp.tile([128, nk, sc], bf16, tag="xc",
                                               name="xcn")
                        nc.sync.dma_start(xcs[nxt][:], xTq[nxt])
                    xc = xcs[ch]
                    # q,k accumulation, k-tile-outer.  Last chunk does k
                    # heads first so kT (needed by the first scores) clears
                    # the rope pipeline early.
                    tlist = (("q", wq_sb, qT_sb), ("k", wk_sb, kT_sb))
                    if ch == nch - 1:
                        tlist = tlist[::-1]
                    for t, w_sb, dstT in tlist:
                        accs = []
                        for h in range(nh_loc):
                            acc = ps1.tile([128, sc], f32, tag="acc")
                            accs.append((acc, h))
                        for k in range(nk):
                            for acc, h in accs:
                                hsl = slice(h * d, (h + 1) * d)
                                nc.tensor.matmul(
                                    acc[:], w_sb[:, k, hsl], xc[:, k, :],
                                    start=(k == 0), stop=(k == nk - 1),
                                )
                        for acc, h in accs:
                            rope(acc, dstT, h, csl)
                    # v for chunks 0-2 (chunk 3 is deferred into phase 2)
                    if ch < nch - 1:
                        for stl in range(sc // 128):
                            st = ch * (sc // 128) + stl
                            ssl = slice(stl * 128, (stl + 1) * 128)
                            accv = ps1.tile([128, hd], f32, tag="acc")
                            for k in range(nk):
                                nc.tensor.matmul(
                                    accv[:], xc[:, k, ssl], wv_sb[:, k, :],
                                    start=(k == 0), stop=(k == nk - 1),
                                )
                            nc.scalar.copy(v_sb[:, st, :], accv[:])

            # ---- phase 2: attention + output projection ------------------
            with tc.tile_pool(name="expp", bufs=2) as expp, \
                 tc.tile_pool(name="tree", bufs=2) as treep, \
                 tc.tile_pool(name="invp", bufs=1) as invp, \
                 tc.tile_pool(name="outp", bufs=4) as outp, \
                 tc.tile_pool(name="ps2s", bufs=2, space="PSUM") as ps2s, \
                 tc.tile_pool(name="ps2p", bufs=2, space="PSUM") as ps2p:

                exs = {}    # i -> exp tile
                roots = {}  # i -> denominator tree root (SBUF bf16)

                def item_S(i):
                    ic, h = divmod(i, nh_loc)
                    ex = expp.tile([128, ns, sw], bf16, tag="exp")
                    exs[i] = ex
                    for jt in range(ns):
                        # weave an oproj unit in every 4 j-tiles: when the
                        # exp backlog paces the scores, this keeps PE busy
                        if jt in (4, 8, 12) and ofill:
                            o_unit()
                        jsl = slice(jt * 128, (jt + 1) * 128)
                        sc_ps = ps2s.tile([128, sw], f32, tag="scores")
                        for half in range(2):
                            qsl = slice(ic * sw + half * sc,
                                        ic * sw + (half + 1) * sc)
                            nc.tensor.matmul(
                                sc_ps[:, half * sc:(half + 1) * sc],
                                kT_sb[:, h, jsl], qT_sb[:, h, qsl],
                                start=True, stop=True)
                        nc.scalar.activation(
                            ex[:, jt, :], sc_ps[:],
                            mybir.ActivationFunctionType.Exp,
                            bias=maskb_sb[:, jt:jt + 1], scale=1.0,
                        )
                    # denominator part 1: running j-tile sum into one tile
                    # (DVE, paced by the exps as they land)
                    u = treep.tile([128, sw], bf16, tag="tree")
                    nc.vector.tensor_add(u[:], ex[:, 0, :], ex[:, 1, :])
                    for a in range(2, ns):
                        nc.vector.tensor_add(u[:], u[:], ex[:, a, :])
                    roots[i] = u

                def item_P(i):
                    ic, h = divmod(i, nh_loc)
                    hsl = slice(h * d, (h + 1) * d)
                    ex = exs.pop(i)
                    root = roots.pop(i)
                    # denominator part 2: sum the 128 j-partitions with one
                    # short ones-matmul per half.  Issued here (not in S)
                    # so the PE doesn't sit on the exp->tree dependency.
                    inv = invp.tile([128, sw], f32, tag="inv")
                    rps = []
                    for half in range(2):
                        fsl = slice(half * sc, (half + 1) * sc)
                        r_ps = psA.tile([128, sc], f32, tag="oproj",
                                         name="r_ps")
                        nc.tensor.matmul(r_ps[:], ones_sb[:, :128], root[:, fsl],
                                         start=True, stop=True)
                        rps.append((r_ps, fsl))
                    for r_ps, fsl in rps:
                        nc.vector.reciprocal_approx_fast(inv[:, fsl], r_ps[:])
                    for half in range(2):
                        fsl = slice(half * sc, (half + 1) * sc)
                        pv_ps = ps2p.tile([128, sc], f32, tag="pv")
                        for jt in range(ns):
                            nc.tensor.matmul(
                                pv_ps[:], v_sb[:, jt, hsl], ex[:, jt, fsl],
                                start=(jt == 0), stop=(jt == ns - 1),
                            )
                        asl = slice(ic * sw + half * sc,
                                    ic * sw + (half + 1) * sc)
                        nc.vector.tensor_mul(attn_sb[:, h, asl],
                                             pv_ps[:], inv[:, fsl])

                def item_V(st):
                    stl = st % (sc // 128)
                    ssl = slice(stl * 128, (stl + 1) * 128)
                    xcv = xc3
                    accv = psA.tile([128, hd], f32, tag="oproj")
                    for k in range(nk):
                        nc.tensor.matmul(
                            accv[:], xcv[:, k, ssl], wv_sb[:, k, :],
                            start=(k == 0), stop=(k == nk - 1),
                        )
                    # DVE, not ScalarE: these run right as the exp pipeline
                    # ramps, and ScalarE is the scarce engine there
                    nc.vector.tensor_copy(v_sb[:, st, :], accv[:])

                # output projection, issued as (et, half) units that fill
                # PE slack between (and inside) the S/P items
                ofill = []
                ocnt = [0]

                def o_unit():
                    ic, et, half = ofill.pop(0)
                    esl = slice(et * 128, (et + 1) * 128)
                    osl = slice(ic * sw + half * sc,
                                ic * sw + (half + 1) * sc)
                    op_ps = psA.tile([128, sc], f32, tag="oproj")
                    for ht in range(nh_loc):
                        nc.tensor.matmul(
                            op_ps[:], wo_sb[:, ht, esl],
                            attn_sb[:, ht, osl],
                            start=(ht == 0), stop=(ht == nh_loc - 1),
                        )
                    ot = outp.tile([128, sc], bf16, tag="ot")
                    # stripe-0 units run while ScalarE is saturated with exp:
                    # drain on DVE only.  Tail units alternate DVE/ScalarE.
                    ocnt[0] += 1
                    if ic == 0 or ocnt[0] % 2 == 0:
                        nc.vector.tensor_copy(ot[:], op_ps[:])
                    else:
                        nc.scalar.copy(ot[:], op_ps[:])
                    nc.sync.dma_start(outT_r[:, et, osl], ot[:])

                v3 = (nch - 1) * (sc // 128)
                # S0 leads (kT clears the rope pipe ~4.5us in; scores jt0
                # waits briefly) so the ScalarE exp pipeline fills ASAP; the
                # scalar-free V items fill the exp-paced region instead of
                # the start, where ScalarE is idle anyway.
                sched = [("V", v3), ("S", 0), ("V", v3 + 1),
                         ("S", 1), ("V", v3 + 2), ("V", v3 + 3),
                         ("P", 0), ("S", 2), ("P", 1),
                         ("S", 3), ("P", 2),
                         ("S", 4), ("P", 3), ("U", 4), ("S", 5), ("U", 2),
                         ("P", 4), ("U", 4), ("S", 6), ("U", 2), ("P", 5),
                         ("U", 4), ("S", 7), ("U", 2), ("P", 6), ("U", 5),
                         ("P", 7)]
                assert nst == 2 and nh_loc == 4
                for kind, arg in sched:
                    if kind == "S":
                        item_S(arg)
                    elif kind == "P":
                        item_P(arg)
                        i = arg
                        if i % nh_loc == nh_loc - 1:
                            ic = i // nh_loc
                            ofill.extend((ic, et, half) for et in range(ne)
                                         for half in range(2))
                    elif kind == "V":
                        item_V(arg)
                    else:
                        for _ in range(min(arg, len(ofill))):
                            o_unit()
                while ofill:
                    o_unit()

    nc.compile()
    return nc


def _get_nc(s=S, dmodel=D, nh_loc=NH_LOC):
    key = (s, dmodel, nh_loc)
    if key not in _NC_CACHE:
        _NC_CACHE[key] = _build_nc(s, dmodel, nh_loc)
    return _NC_CACHE[key]


def _rope_tables(s, d, dtype=np.float32):
    inv_freq = 1.0 / (ROPE_THETA ** (np.arange(0, d, 2, dtype=np.float64) / d))
    pos = np.arange(s, dtype=np.float64)
    freqs = pos[:, None] * inv_freq[None, :]            # [s, d/2]
    emb = np.concatenate([freqs, freqs], axis=-1)       # [s, d]
    return np.cos(emb).astype(dtype), np.sin(emb).astype(dtype)


def make_in_maps(hidden_states, sequence_mask, Wqkv, Wo,
                 s=S, b=B, dmodel=D, nh_tot=N_HEADS, nh_loc=NH_LOC, d=DQK):
    bf = ml_dtypes.bfloat16
    cos, sin = _rope_tables(s, d)
    cosT = np.ascontiguousarray(cos.T).astype(bf)       # [d, s]
    sinT = np.ascontiguousarray(sin.T)                  # [d, s] f32
    ssh = sinT.copy()
    ssh[d // 2:] = -ssh[d // 2:]
    sshT = ssh.astype(bf)
    scale = 1.0 / np.sqrt(np.float32(d))

    in_maps = []
    cores_per_batch = N_CORES // b
    for c in range(N_CORES):
        bi = c // cores_per_batch
        g = c % cores_per_batch
        h0 = g * nh_loc
        hsl = slice(h0 * d, (h0 + nh_loc) * d)
        nk, sc, nch = dmodel // 128, 512, s // 512
        xb = hidden_states[:, bi, :]                    # [s, dmodel]
        # [nch, 128, nk, sc]: xTq[ch, p, k, j] = x[ch*sc+j, k*128+p]
        xTq = np.ascontiguousarray(
            xb.T.reshape(nk, 128, nch, sc).transpose(2, 1, 0, 3)).astype(bf)
        hd = nh_loc * d

        def swz_w(w):   # [dmodel, hd] -> [128, nk, hd]
            return np.ascontiguousarray(
                w.reshape(nk, 128, hd).transpose(1, 0, 2)).astype(bf)

        wq = swz_w(Wqkv[:, 0 * nh_tot * d:1 * nh_tot * d][:, hsl] * scale)
        wk = swz_w(Wqkv[:, 1 * nh_tot * d:2 * nh_tot * d][:, hsl])
        wv = swz_w(Wqkv[:, 2 * nh_tot * d:3 * nh_tot * d][:, hsl])
        # [128, nh_loc, dmodel]: wo[p, h, e] = Wo[h*128+p, e]
        wo = np.ascontiguousarray(
            Wo[hsl, :].reshape(nh_loc, 128, dmodel).transpose(1, 0, 2)
        ).astype(bf)
        bias = np.where(sequence_mask[bi] == 0, -1e30, 0.0).astype(np.float32)
        maskbT = np.ascontiguousarray(bias.reshape(s // 128, 128).T)  # [128, ns]
        in_maps.append({
            "xTq": xTq, "wq": wq, "wk": wk, "wv": wv, "wo": wo,
            "cosT": cosT, "sshT": sshT, "maskb": maskbT,
        })
    return in_maps


def kernel(hidden_states, sequence_mask, Wqkv, Wo):
    global LAST_RESULT
    from concourse.bass_utils import run_bass_kernel_spmd

    hidden_states = np.asarray(hidden_states)
    sequence_mask = np.asarray(sequence_mask)
    Wqkv = np.asarray(Wqkv)
    Wo = np.asarray(Wo)

    nc = _get_nc()
    in_maps = make_in_maps(hidden_states, sequence_mask, Wqkv, Wo)
    res = run_bass_kernel_spmd(
        nc, in_maps, list(range(N_CORES)),
        trace=bool(int(os.environ.get("KERNEL_TRACE", "0"))),
    )
    LAST_RESULT = res

    out = np.empty((S, B, D), dtype=np.float32)
    cores_per_batch = N_CORES // B
    for bi in range(B):
        acc = None
        for g in range(cores_per_batch):
            part = res.results[bi * cores_per_batch + g]["outT"]  # [D, S] bf16
            part = np.asarray(part, dtype=np.float32)
            acc = part if acc is None else acc + part
        out[:, bi, :] = acc.T
    return out


# revision 22
# speedup vs baseline: 1.0330x; 1.0065x over previous
"""Trainium2 Bass kernel for 16-head RoPE self-attention (S=2048, B=2, D=2048).

Sharding: 8 cores = 2 batches x 4 head-groups (4 heads each). Each core
computes qkv projection for its batch/heads, full attention over its 4
heads, and a partial output projection (its 4-head slice of Wo rows).
Host sums the 4 partial outputs per batch. No cross-core collectives.

The kernel is PE-bound (~92% tensor busy), so every change targets PE
busy-cycles or PE idle gaps:
  - warmup block: 16 dummy 512-row matmuls on a memset tile at kernel
    start keep the PE HAM activity window busy during the initial
    weight/x DMA, so the clock gate opens at ~11us instead of ~25us.
  - qkv projection runs k-tile-outer x head-inner so the first chunk's
    accumulation consumes weight/x tiles in DMA arrival order.
  - RoPE's rotate_half is done with partition-shifted DVE multiplies
    (reading acc straight from PSUM via a bf16 copy) instead of a
    128x128 permutation matmul: -16k PE cycles.
  - softmax denominator: full pairwise DVE tree to one tile, then one
    short ones-matmul per half (issued inside the P item so the PE never
    waits on the exp->tree chain): -25k PE cycles vs v1.
  - phase 2 is a single software-pipelined worklist: scores(i) issue two
    heads ahead of PV(i) so the ScalarE exp backlog never stalls the PE;
    the 4th x-chunk's v-projection and the previous stripe's output
    projection pieces fill the remaining gaps.
  - output is written bf16 (host upcasts + sums partials): halves the
    out-DMA so the tail oproj isn't DMA-limited.
"""

import os
import numpy as np
import ml_dtypes

S, B, D = 2048, 2, 2048
N_HEADS, DQK = 16, 128
ROPE_THETA = 500000.0
N_CORES = 8
CORES_PER_BATCH = 4
NH_LOC = N_HEADS // CORES_PER_BATCH  # 4 heads per core

LAST_RESULT = None  # BassKernelResults of the most recent run (for test.py)

_NC_CACHE = {}


def _build_nc(s, dmodel, nh_loc, d=DQK, sc=512):
    import concourse.tile as tile
    from concourse import bacc, mybir

    bf16 = mybir.dt.bfloat16
    f32 = mybir.dt.float32
    nk = dmodel // 128      # contraction tiles for the projections
    ns = s // 128           # sequence tiles (key side)
    nch = s // sc           # sequence chunks (query side / moving dim)
    hd = nh_loc * d         # local head-dim total (512)
    ne = dmodel // 128      # output-embedding tiles
    sw = 2 * sc             # query stripe width (1024)
    nst = s // sw           # stripes (2)
    h2 = d // 2             # rotate_half split (64)

    nc = bacc.Bacc("TRN2", target_bir_lowering=False, debug=False)
    # inputs are pre-swizzled on the host to partition-major layouts so DMA
    # descriptors are 4-16KB contiguous runs instead of 1KB
    xTq = nc.dram_tensor("xTq", [nch, 128, nk, sc], bf16, kind="ExternalInput")
    wq = nc.dram_tensor("wq", [128, nk, hd], bf16, kind="ExternalInput")
    wk = nc.dram_tensor("wk", [128, nk, hd], bf16, kind="ExternalInput")
    wv = nc.dram_tensor("wv", [128, nk, hd], bf16, kind="ExternalInput")
    wo = nc.dram_tensor("wo", [128, hd // 128, dmodel], bf16,
                        kind="ExternalInput")
    cosT = nc.dram_tensor("cosT", [d, s], bf16, kind="ExternalInput")
    # sshT = sin.T with the BOTTOM 64 rows negated; the multiply for output
    # rows [0:64) reads ssh rows [64:128) (= -sin, base-partition-aligned
    # with raw[64:128)) and vice versa: rot(q)[p]*sin[p] = raw[p^64]*ssh[p^64]
    sshT = nc.dram_tensor("sshT", [d, s], bf16, kind="ExternalInput")
    maskb = nc.dram_tensor("maskb", [128, ns], f32, kind="ExternalInput")
    outT = nc.dram_tensor("outT", [dmodel, s], bf16, kind="ExternalOutput")

    outT_r = outT.rearrange("(e p) s -> p e s", p=128)

    with tile.TileContext(nc) as tc:
        with tc.tile_pool(name="const", bufs=1) as constp, \
             tc.tile_pool(name="store", bufs=1) as storep, \
             tc.tile_pool(name="psA", bufs=2, space="PSUM") as psA:
            qT_sb = storep.tile([128, nh_loc, s], bf16)
            kT_sb = storep.tile([128, nh_loc, s], bf16)
            v_sb = storep.tile([128, ns, hd], bf16)
            attn_sb = storep.tile([128, nh_loc, s], bf16)
            xc3 = storep.tile([128, nk, sc], bf16)  # chunk-3 x, used in ph2

            # ---- phase 1: q,k projection + rope; v for chunks 0-2 --------
            with tc.tile_pool(name="wqk", bufs=1) as wp, \
                 tc.tile_pool(name="tab", bufs=1) as tabp, \
                 tc.tile_pool(name="xch", bufs=2) as xp, \
                 tc.tile_pool(name="rope", bufs=2) as rp, \
                 tc.tile_pool(name="ps1", bufs=6, space="PSUM") as ps1:
                # per-k-tile DMAs in exactly the order the k-outer
                # accumulation consumes them
                wq_sb = wp.tile([128, nk, hd], bf16, tag="wq")
                wk_sb = wp.tile([128, nk, hd], bf16, tag="wk")
                xc0 = xp.tile([128, nk, sc], bf16, tag="xc")
                for k in range(0, nk, 2):
                    ks = slice(k, k + 2)
                    nc.sync.dma_start(wq_sb[:, ks, :], wq[:, ks, :])
                    nc.sync.dma_start(xc0[:, ks, :], xTq[0, :, ks, :])
                for k in range(0, nk, 4):
                    ks = slice(k, k + 4)
                    nc.sync.dma_start(wk_sb[:, ks, :], wk[:, ks, :])
                cos_sb = tabp.tile([128, s], bf16)
                nc.sync.dma_start(cos_sb[:], cosT[:])
                ssh_sb = tabp.tile([128, s], bf16)
                nc.sync.dma_start(ssh_sb[:], sshT[:])
                maskb_sb = constp.tile([128, ns], f32)
                nc.sync.dma_start(maskb_sb[:], maskb[:])
                wv_sb = constp.tile([128, nk, hd], bf16)
                gw = 4
                for g in range(0, nk, gw):
                    gs = slice(g, g + gw)
                    nc.sync.dma_start(wv_sb[:, gs, :], wv[:, gs, :])
                # chunk-1 x before wo: chunk 1's first matmul needs it ~45us
                # in, wo isn't needed until ~150us
                xcs = {0: xc0}
                xcs[1] = xp.tile([128, nk, sc], bf16, tag="xc", name="xc1")
                nc.sync.dma_start(xcs[1][:], xTq[1])
                wo_sb = constp.tile([128, nh_loc, dmodel], bf16)
                nc.sync.dma_start(wo_sb[:], wo[:])

                # warmup: keep the PE busy while the first DMAs land so the
                # HAM clock gate opens early.  The operands come from a memset
                # (no DMA dependency -> starts at ~0.5us); 512-row matmuls
                # keep the PE duty cycle high (the paired LDWEIGHTS is only
                # 128 rows), round-robining 4 PSUM slots to avoid WAW stalls.
                # ones_sb doubles as the denominator's summing stationary.
                # the warmup matmuls read ones_sb BEFORE the memset: their
                # output is never read, so uninitialized data is fine, and
                # skipping the memset wait lets the PE start ~2.5us earlier.
                # The memset (WAR-ordered after the last warm matmul) then
                # fills the real all-ones operand for the denominator sums.
                ones_sb = constp.tile([128, sc], bf16)
                warms = []
                for _ in range(4):
                    warm = ps1.tile([128, sc], f32, tag="acc", name="warm")
                    warms.append(warm)
                for w in range(12):
                    warm = warms[w % 4]
                    nc.tensor.matmul(warm[:], ones_sb[:, :128], ones_sb[:],
                                     start=True, stop=True)
                nc.vector.memset(ones_sb[:], 1.0)

                def rope(acc, dstT, h, csl):
                    raw = rp.tile([128, sc], bf16, tag="raw")
                    nc.scalar.copy(raw[:], acc[:])
                    t1 = rp.tile([128, sc], bf16, tag="t1")
                    nc.vector.tensor_mul(t1[:], raw[:], cos_sb[:, csl])
                    t2 = rp.tile([128, sc], bf16, tag="t2")
                    # both SBUF inputs of a TensorTensor must share a base
                    # partition, so each multiply uses the ssh rows aligned
                    # with its raw slice (sin rows repeat: sin[m]==sin[m+64])
                    nc.vector.tensor_mul(t2[:h2, :], raw[h2:, :],
                                         ssh_sb[h2:, csl])
                    nc.vector.tensor_mul(t2[h2:, :], raw[:h2, :],
                                         ssh_sb[:h2, csl])
                    nc.vector.tensor_add(dstT[:, h, csl], t1[:], t2[:])

                for ch in range(nch):
                    csl = slice(ch * sc, (ch + 1) * sc)
                    # prefetch next chunk's x (chunks 0,1 pre-issued above)
                    nxt = ch + 1
                    if nxt < nch and nxt not in xcs:
                        if nxt == nch - 1:
                            xcs[nxt] = xc3
                        else:
                            xcs[nxt] = xp.tile([128, nk, sc], bf16, tag="xc",
                                               name="xcn")
                        nc.sync.dma_start(xcs[nxt][:], xTq[nxt])
                    xc = xcs[ch]
                    # q,k accumulation, k-tile-outer.  Last chunk does k
                    # heads first so kT (needed by the first scores) clears
                    # the rope pipeline early.
                    tlist = (("q", wq_sb, qT_sb), ("k", wk_sb, kT_sb))
                    if ch == nch - 1:
                        tlist = tlist[::-1]
                    for t, w_sb, dstT in tlist:
                        accs = []
                        for h in range(nh_loc):
                            acc = ps1.tile([128, sc], f32, tag="acc")
                            accs.append((acc, h))
                        for k in range(nk):
                            for acc, h in accs:
                                hsl = slice(h * d, (h + 1) * d)
                                nc.tensor.matmul(
                                    acc[:], w_sb[:, k, hsl], xc[:, k, :],
                                    start=(k == 0), stop=(k == nk - 1),
                                )
                        for acc, h in accs:
                            rope(acc, dstT, h, csl)
                    # v for chunks 0-2 (chunk 3 is deferred into phase 2)
                    if ch < nch - 1:
                        for stl in range(sc // 128):
                            st = ch * (sc // 128) + stl
                            ssl = slice(stl * 128, (stl + 1) * 128)
                            accv = ps1.tile([128, hd], f32, tag="acc")
                            for k in range(nk):
                                nc.tensor.matmul(
                                    accv[:], xc[:, k, ssl], wv_sb[:, k, :],
                                    start=(k == 0), stop=(k == nk - 1),
                                )
                            nc.scalar.copy(v_sb[:, st, :], accv[:])

            # ---- phase 2: attention + output projection ------------------
            with tc.tile_pool(name="expp", bufs=2) as expp, \
                 tc.tile_pool(name="tree", bufs=2) as treep, \
                 tc.tile_pool(name="invp", bufs=1) as invp, \
                 tc.tile_pool(name="outp", bufs=8) as outp, \
                 tc.tile_pool(name="ps2s", bufs=2, space="PSUM") as ps2s, \
                 tc.tile_pool(name="ps2p", bufs=2, space="PSUM") as ps2p:

                exs = {}    # i -> exp tile
                roots = {}  # i -> denominator tree root (SBUF bf16)

                def item_S(i):
                    ic, h = divmod(i, nh_loc)
                    ex = expp.tile([128, ns, sw], bf16, tag="exp")
                    exs[i] = ex
                    for jt in range(ns):
                        # weave an oproj unit in every 4 j-tiles: when the
                        # exp backlog paces the scores, this keeps PE busy
                        if jt in (4, 8, 12) and ofill:
                            o_unit()
                        jsl = slice(jt * 128, (jt + 1) * 128)
                        sc_ps = ps2s.tile([128, sw], f32, tag="scores")
                        for half in range(2):
                            qsl = slice(ic * sw + half * sc,
                                        ic * sw + (half + 1) * sc)
                            nc.tensor.matmul(
                                sc_ps[:, half * sc:(half + 1) * sc],
                                kT_sb[:, h, jsl], qT_sb[:, h, qsl],
                                start=True, stop=True)
                        nc.scalar.activation(
                            ex[:, jt, :], sc_ps[:],
                            mybir.ActivationFunctionType.Exp,
                            bias=maskb_sb[:, jt:jt + 1], scale=1.0,
                        )
                    # denominator part 1: running j-tile sum into one tile
                    # (DVE, paced by the exps as they land)
                    u = treep.tile([128, sw], bf16, tag="tree")
                    nc.vector.tensor_add(u[:], ex[:, 0, :], ex[:, 1, :])
                    for a in range(2, ns):
                        nc.vector.tensor_add(u[:], u[:], ex[:, a, :])
                    roots[i] = u

                def item_P(i):
                    ic, h = divmod(i, nh_loc)
                    hsl = slice(h * d, (h + 1) * d)
                    ex = exs.pop(i)
                    root = roots.pop(i)
                    # denominator part 2: sum the 128 j-partitions with one
                    # short ones-matmul per half.  Issued here (not in S)
                    # so the PE doesn't sit on the exp->tree dependency.
                    inv = invp.tile([128, sw], f32, tag="inv")
                    rps = []
                    for half in range(2):
                        fsl = slice(half * sc, (half + 1) * sc)
                        r_ps = psA.tile([128, sc], f32, tag="oproj",
                                         name="r_ps")
                        nc.tensor.matmul(r_ps[:], ones_sb[:, :128], root[:, fsl],
                                         start=True, stop=True)
                        rps.append((r_ps, fsl))
                    for r_ps, fsl in rps:
                        nc.vector.reciprocal_approx_fast(inv[:, fsl], r_ps[:])
                    for half in range(2):
                        fsl = slice(half * sc, (half + 1) * sc)
                        pv_ps = ps2p.tile([128, sc], f32, tag="pv")
                        for jt in range(ns):
                            nc.tensor.matmul(
                                pv_ps[:], v_sb[:, jt, hsl], ex[:, jt, fsl],
                                start=(jt == 0), stop=(jt == ns - 1),
                            )
                        asl = slice(ic * sw + half * sc,
                                    ic * sw + (half + 1) * sc)
                        nc.vector.tensor_mul(attn_sb[:, h, asl],
                                             pv_ps[:], inv[:, fsl])

                def item_V(st):
                    stl = st % (sc // 128)
                    ssl = slice(stl * 128, (stl + 1) * 128)
                    xcv = xc3
                    accv = psA.tile([128, hd], f32, tag="oproj")
                    for k in range(nk):
                        nc.tensor.matmul(
                            accv[:], xcv[:, k, ssl], wv_sb[:, k, :],
                            start=(k == 0), stop=(k == nk - 1),
                        )
                    # DVE, not ScalarE: these run right as the exp pipeline
                    # ramps, and ScalarE is the scarce engine there
                    nc.vector.tensor_copy(v_sb[:, st, :], accv[:])

                # output projection, issued as (et, half) units that fill
                # PE slack between (and inside) the S/P items
                ofill = []
                ocnt = [0]

                def o_unit():
                    ic, et, half = ofill.pop(0)
                    esl = slice(et * 128, (et + 1) * 128)
                    osl = slice(ic * sw + half * sc,
                                ic * sw + (half + 1) * sc)
                    op_ps = psA.tile([128, sc], f32, tag="oproj")
                    for ht in range(nh_loc):
                        nc.tensor.matmul(
                            op_ps[:], wo_sb[:, ht, esl],
                            attn_sb[:, ht, osl],
                            start=(ht == 0), stop=(ht == nh_loc - 1),
                        )
                    ot = outp.tile([128, sc], bf16, tag="ot")
                    # stripe-0 units run while ScalarE is saturated with exp:
                    # drain on DVE only.  Tail units alternate DVE/ScalarE.
                    ocnt[0] += 1
                    if ic == 0 or ocnt[0] % 2 == 0:
                        nc.vector.tensor_copy(ot[:], op_ps[:])
                    else:
                        nc.scalar.copy(ot[:], op_ps[:])
                    nc.sync.dma_start(outT_r[:, et, osl], ot[:])

                v3 = (nch - 1) * (sc // 128)
                # S0 leads (kT clears the rope pipe ~4.5us in; scores jt0
                # waits briefly) so the ScalarE exp pipeline fills ASAP; the
                # scalar-free V items fill the exp-paced region instead of
                # the start, where ScalarE is idle anyway.
                sched = [("V", v3), ("S", 0), ("V", v3 + 1),
                         ("S", 1), ("V", v3 + 2), ("V", v3 + 3),
                         ("P", 0), ("S", 2), ("P", 1),
                         ("S", 3), ("P", 2),
                         ("S", 4), ("P", 3), ("U", 4), ("S", 5), ("U", 2),
                         ("P", 4), ("U", 4), ("S", 6), ("U", 2), ("P", 5),
                         ("U", 4), ("S", 7), ("U", 2), ("P", 6), ("U", 5),
                         ("P", 7)]
                assert nst == 2 and nh_loc == 4
                for kind, arg in sched:
                    if kind == "S":
                        item_S(arg)
                    elif kind == "P":
                        item_P(arg)
                        i = arg
                        if i % nh_loc == nh_loc - 1:
                            ic = i // nh_loc
                            ofill.extend((ic, et, half) for et in range(ne)
                                         for half in range(2))
                    elif kind == "V":
                        item_V(arg)
                    else:
                        for _ in range(min(arg, len(ofill))):
                            o_unit()
                while ofill:
                    o_unit()

    nc.compile()
    return nc


def _get_nc(s=S, dmodel=D, nh_loc=NH_LOC):
    key = (s, dmodel, nh_loc)
    if key not in _NC_CACHE:
        _NC_CACHE[key] = _build_nc(s, dmodel, nh_loc)
    return _NC_CACHE[key]


def _rope_tables(s, d, dtype=np.float32):
    inv_freq = 1.0 / (ROPE_THETA ** (np.arange(0, d, 2, dtype=np.float64) / d))
    pos = np.arange(s, dtype=np.float64)
    freqs = pos[:, None] * inv_freq[None, :]            # [s, d/2]
    emb = np.concatenate([freqs, freqs], axis=-1)       # [s, d]
    return np.cos(emb).astype(dtype), np.sin(emb).astype(dtype)


def make_in_maps(hidden_states, sequence_mask, Wqkv, Wo,
                 s=S, b=B, dmodel=D, nh_tot=N_HEADS, nh_loc=NH_LOC, d=DQK):
    bf = ml_dtypes.bfloat16
    cos, sin = _rope_tables(s, d)
    cosT = np.ascontiguousarray(cos.T).astype(bf)       # [d, s]
    sinT = np.ascontiguousarray(sin.T)                  # [d, s] f32
    ssh = sinT.copy()
    ssh[d // 2:] = -ssh[d // 2:]
    sshT = ssh.astype(bf)
    scale = 1.0 / np.sqrt(np.float32(d))

    in_maps = []
    cores_per_batch = N_CORES // b
    for c in range(N_CORES):
        bi = c // cores_per_batch
        g = c % cores_per_batch
        h0 = g * nh_loc
        hsl = slice(h0 * d, (h0 + nh_loc) * d)
        nk, sc, nch = dmodel // 128, 512, s // 512
        xb = hidden_states[:, bi, :]                    # [s, dmodel]
        # [nch, 128, nk, sc]: xTq[ch, p, k, j] = x[ch*sc+j, k*128+p]
        xTq = np.ascontiguousarray(
            xb.T.reshape(nk, 128, nch, sc).transpose(2, 1, 0, 3)).astype(bf)
        hd = nh_loc * d

        def swz_w(w):   # [dmodel, hd] -> [128, nk, hd]
            return np.ascontiguousarray(
                w.reshape(nk, 128, hd).transpose(1, 0, 2)).astype(bf)

        wq = swz_w(Wqkv[:, 0 * nh_tot * d:1 * nh_tot * d][:, hsl] * scale)
        wk = swz_w(Wqkv[:, 1 * nh_tot * d:2 * nh_tot * d][:, hsl])
        wv = swz_w(Wqkv[:, 2 * nh_tot * d:3 * nh_tot * d][:, hsl])
        # [128, nh_loc, dmodel]: wo[p, h, e] = Wo[h*128+p, e]
        wo = np.ascontiguousarray(
            Wo[hsl, :].reshape(nh_loc, 128, dmodel).transpose(1, 0, 2)
        ).astype(bf)
        bias = np.where(sequence_mask[bi] == 0, -1e30, 0.0).astype(np.float32)
        maskbT = np.ascontiguousarray(bias.reshape(s // 128, 128).T)  # [128, ns]
        in_maps.append({
            "xTq": xTq, "wq": wq, "wk": wk, "wv": wv, "wo": wo,
            "cosT": cosT, "sshT": sshT, "maskb": maskbT,
        })
    return in_maps


def kernel(hidden_states, sequence_mask, Wqkv, Wo):
    global LAST_RESULT
    from concourse.bass_utils import run_bass_kernel_spmd

    hidden_states = np.asarray(hidden_states)
    sequence_mask = np.asarray(sequence_mask)
    Wqkv = np.asarray(Wqkv)
    Wo = np.asarray(Wo)

    nc = _get_nc()
    in_maps = make_in_maps(hidden_states, sequence_mask, Wqkv, Wo)
    res = run_bass_kernel_spmd(
        nc, in_maps, list(range(N_CORES)),
        trace=bool(int(os.environ.get("KERNEL_TRACE", "0"))),
    )
    LAST_RESULT = res

    out = np.empty((S, B, D), dtype=np.float32)
    cores_per_batch = N_CORES // B
    for bi in range(B):
        acc = None
        for g in range(cores_per_batch):
            part = res.results[bi * cores_per_batch + g]["outT"]  # [D, S] bf16
            part = np.asarray(part, dtype=np.float32)
            acc = part if acc is None else acc + part
        out[:, bi, :] = acc.T
    return out


# revision 23
# speedup vs baseline: 1.0375x; 1.0043x over previous
"""Trainium2 Bass kernel for 16-head RoPE self-attention (S=2048, B=2, D=2048).

Sharding: 8 cores = 2 batches x 4 head-groups (4 heads each). Each core
computes qkv projection for its batch/heads, full attention over its 4
heads, and a partial output projection (its 4-head slice of Wo rows).
Host sums the 4 partial outputs per batch. No cross-core collectives.

The kernel is PE-bound (~92% tensor busy), so every change targets PE
busy-cycles or PE idle gaps:
  - warmup block: 16 dummy 512-row matmuls on a memset tile at kernel
    start keep the PE HAM activity window busy during the initial
    weight/x DMA, so the clock gate opens at ~11us instead of ~25us.
  - qkv projection runs k-tile-outer x head-inner so the first chunk's
    accumulation consumes weight/x tiles in DMA arrival order.
  - RoPE's rotate_half is done with partition-shifted DVE multiplies
    (reading acc straight from PSUM via a bf16 copy) instead of a
    128x128 permutation matmul: -16k PE cycles.
  - softmax denominator: full pairwise DVE tree to one tile, then one
    short ones-matmul per half (issued inside the P item so the PE never
    waits on the exp->tree chain): -25k PE cycles vs v1.
  - phase 2 is a single software-pipelined worklist: scores(i) issue two
    heads ahead of PV(i) so the ScalarE exp backlog never stalls the PE;
    the 4th x-chunk's v-projection and the previous stripe's output
    projection pieces fill the remaining gaps.
  - output is written bf16 (host upcasts + sums partials): halves the
    out-DMA so the tail oproj isn't DMA-limited.
"""

import os
import numpy as np
import ml_dtypes

S, B, D = 2048, 2, 2048
N_HEADS, DQK = 16, 128
ROPE_THETA = 500000.0
N_CORES = 8
CORES_PER_BATCH = 4
NH_LOC = N_HEADS // CORES_PER_BATCH  # 4 heads per core

LAST_RESULT = None  # BassKernelResults of the most recent run (for test.py)

_NC_CACHE = {}


def _build_nc(s, dmodel, nh_loc, d=DQK, sc=512):
    import concourse.tile as tile
    from concourse import bacc, mybir

    bf16 = mybir.dt.bfloat16
    f32 = mybir.dt.float32
    nk = dmodel // 128      # contraction tiles for the projections
    ns = s // 128           # sequence tiles (key side)
    nch = s // sc           # sequence chunks (query side / moving dim)
    hd = nh_loc * d         # local head-dim total (512)
    ne = dmodel // 128      # output-embedding tiles
    sw = 2 * sc             # query stripe width (1024)
    nst = s // sw           # stripes (2)
    h2 = d // 2             # rotate_half split (64)

    nc = bacc.Bacc("TRN2", target_bir_lowering=False, debug=False)
    # inputs are pre-swizzled on the host to partition-major layouts so DMA
    # descriptors are 4-16KB contiguous runs instead of 1KB
    xTq = nc.dram_tensor("xTq", [nch, 128, nk, sc], bf16, kind="ExternalInput")
    wq = nc.dram_tensor("wq", [128, nk, hd], bf16, kind="ExternalInput")
    wk = nc.dram_tensor("wk", [128, nk, hd], bf16, kind="ExternalInput")
    wv = nc.dram_tensor("wv", [128, nk, hd], bf16, kind="ExternalInput")
    wo = nc.dram_tensor("wo", [128, hd // 128, dmodel], bf16,
                        kind="ExternalInput")
    cosT = nc.dram_tensor("cosT", [d, s], bf16, kind="ExternalInput")
    # sshT = sin.T with the BOTTOM 64 rows negated; the multiply for output
    # rows [0:64) reads ssh rows [64:128) (= -sin, base-partition-aligned
    # with raw[64:128)) and vice versa: rot(q)[p]*sin[p] = raw[p^64]*ssh[p^64]
    sshT = nc.dram_tensor("sshT", [d, s], bf16, kind="ExternalInput")
    maskb = nc.dram_tensor("maskb", [128, ns], f32, kind="ExternalInput")
    outT = nc.dram_tensor("outT", [dmodel, s], bf16, kind="ExternalOutput")

    outT_r = outT.rearrange("(e p) s -> p e s", p=128)

    with tile.TileContext(nc) as tc:
        with tc.tile_pool(name="const", bufs=1) as constp, \
             tc.tile_pool(name="store", bufs=1) as storep, \
             tc.tile_pool(name="psA", bufs=2, space="PSUM") as psA:
            qT_sb = storep.tile([128, nh_loc, s], bf16)
            kT_sb = storep.tile([128, nh_loc, s], bf16)
            v_sb = storep.tile([128, ns, hd], bf16)
            attn_sb = storep.tile([128, nh_loc, s], bf16)
            xc3 = storep.tile([128, nk, sc], bf16)  # chunk-3 x, used in ph2

            # ---- phase 1: q,k projection + rope; v for chunks 0-2 --------
            with tc.tile_pool(name="wqk", bufs=1) as wp, \
                 tc.tile_pool(name="tab", bufs=1) as tabp, \
                 tc.tile_pool(name="xch", bufs=2) as xp, \
                 tc.tile_pool(name="rope", bufs=2) as rp, \
                 tc.tile_pool(name="ps1", bufs=6, space="PSUM") as ps1:
                # per-k-tile DMAs in exactly the order the k-outer
                # accumulation consumes them
                wq_sb = wp.tile([128, nk, hd], bf16, tag="wq")
                wk_sb = wp.tile([128, nk, hd], bf16, tag="wk")
                xc0 = xp.tile([128, nk, sc], bf16, tag="xc")
                for k in range(0, nk, 2):
                    ks = slice(k, k + 2)
                    nc.sync.dma_start(wq_sb[:, ks, :], wq[:, ks, :])
                    nc.sync.dma_start(xc0[:, ks, :], xTq[0, :, ks, :])
                for k in range(0, nk, 4):
                    ks = slice(k, k + 4)
                    nc.sync.dma_start(wk_sb[:, ks, :], wk[:, ks, :])
                cos_sb = tabp.tile([128, s], bf16)
                nc.sync.dma_start(cos_sb[:], cosT[:])
                ssh_sb = tabp.tile([128, s], bf16)
                nc.sync.dma_start(ssh_sb[:], sshT[:])
                maskb_sb = constp.tile([128, ns], f32)
                nc.sync.dma_start(maskb_sb[:], maskb[:])
                wv_sb = constp.tile([128, nk, hd], bf16)
                gw = 4
                for g in range(0, nk, gw):
                    gs = slice(g, g + gw)
                    nc.sync.dma_start(wv_sb[:, gs, :], wv[:, gs, :])
                # chunk-1 x before wo: chunk 1's first matmul needs it ~45us
                # in, wo isn't needed until ~150us
                xcs = {0: xc0}
                xcs[1] = xp.tile([128, nk, sc], bf16, tag="xc", name="xc1")
                nc.sync.dma_start(xcs[1][:], xTq[1])
                wo_sb = constp.tile([128, nh_loc, dmodel], bf16)
                nc.sync.dma_start(wo_sb[:], wo[:])

                # warmup: keep the PE busy while the first DMAs land so the
                # HAM clock gate opens early.  The operands come from a memset
                # (no DMA dependency -> starts at ~0.5us); 512-row matmuls
                # keep the PE duty cycle high (the paired LDWEIGHTS is only
                # 128 rows), round-robining 4 PSUM slots to avoid WAW stalls.
                # ones_sb doubles as the denominator's summing stationary.
                # the warmup matmuls read ones_sb BEFORE the memset: their
                # output is never read, so uninitialized data is fine, and
                # skipping the memset wait lets the PE start ~2.5us earlier.
                # The memset (WAR-ordered after the last warm matmul) then
                # fills the real all-ones operand for the denominator sums.
                ones_sb = constp.tile([128, sc], bf16)
                warms = []
                for _ in range(4):
                    warm = ps1.tile([128, sc], f32, tag="acc", name="warm")
                    warms.append(warm)
                for w in range(12):
                    warm = warms[w % 4]
                    nc.tensor.matmul(warm[:], ones_sb[:, :128], ones_sb[:],
                                     start=True, stop=True)
                nc.vector.memset(ones_sb[:], 1.0)

                def rope(acc, dstT, h, csl):
                    raw = rp.tile([128, sc], bf16, tag="raw")
                    nc.scalar.copy(raw[:], acc[:])
                    t1 = rp.tile([128, sc], bf16, tag="t1")
                    nc.vector.tensor_mul(t1[:], raw[:], cos_sb[:, csl])
                    t2 = rp.tile([128, sc], bf16, tag="t2")
                    # both SBUF inputs of a TensorTensor must share a base
                    # partition, so each multiply uses the ssh rows aligned
                    # with its raw slice (sin rows repeat: sin[m]==sin[m+64])
                    nc.vector.tensor_mul(t2[:h2, :], raw[h2:, :],
                                         ssh_sb[h2:, csl])
                    nc.vector.tensor_mul(t2[h2:, :], raw[:h2, :],
                                         ssh_sb[:h2, csl])
                    nc.vector.tensor_add(dstT[:, h, csl], t1[:], t2[:])

                for ch in range(nch):
                    csl = slice(ch * sc, (ch + 1) * sc)
                    # prefetch next chunk's x (chunks 0,1 pre-issued above)
                    nxt = ch + 1
                    if nxt < nch and nxt not in xcs:
                        if nxt == nch - 1:
                            xcs[nxt] = xc3
                        else:
                            xcs[nxt] = xp.tile([128, nk, sc], bf16, tag="xc",
                                               name="xcn")
                        nc.sync.dma_start(xcs[nxt][:], xTq[nxt])
                    xc = xcs[ch]
                    # q,k accumulation, k-tile-outer.  Last chunk does k
                    # heads first so kT (needed by the first scores) clears
                    # the rope pipeline early.
                    tlist = (("q", wq_sb, qT_sb), ("k", wk_sb, kT_sb))
                    if ch == nch - 1:
                        tlist = tlist[::-1]
                    for t, w_sb, dstT in tlist:
                        accs = []
                        for h in range(nh_loc):
                            acc = ps1.tile([128, sc], f32, tag="acc")
                            accs.append((acc, h))
                        for k in range(nk):
                            for acc, h in accs:
                                hsl = slice(h * d, (h + 1) * d)
                                nc.tensor.matmul(
                                    acc[:], w_sb[:, k, hsl], xc[:, k, :],
                                    start=(k == 0), stop=(k == nk - 1),
                                )
                        for acc, h in accs:
                            rope(acc, dstT, h, csl)
                    # v for chunks 0-2 (chunk 3 is deferred into phase 2)
                    if ch < nch - 1:
                        for stl in range(sc // 128):
                            st = ch * (sc // 128) + stl
                            ssl = slice(stl * 128, (stl + 1) * 128)
                            accv = ps1.tile([128, hd], f32, tag="acc")
                            for k in range(nk):
                                nc.tensor.matmul(
                                    accv[:], xc[:, k, ssl], wv_sb[:, k, :],
                                    start=(k == 0), stop=(k == nk - 1),
                                )
                            nc.scalar.copy(v_sb[:, st, :], accv[:])

            # ---- phase 2: attention + output projection ------------------
            with tc.tile_pool(name="expp", bufs=2) as expp, \
                 tc.tile_pool(name="tree", bufs=2) as treep, \
                 tc.tile_pool(name="invp", bufs=1) as invp, \
                 tc.tile_pool(name="outp", bufs=8) as outp, \
                 tc.tile_pool(name="ps2s", bufs=2, space="PSUM") as ps2s, \
                 tc.tile_pool(name="ps2p", bufs=2, space="PSUM") as ps2p:

                exs = {}    # i -> exp tile
                roots = {}  # i -> denominator tree root (SBUF bf16)

                def item_S(i):
                    ic, h = divmod(i, nh_loc)
                    ex = expp.tile([128, ns, sw], bf16, tag="exp")
                    exs[i] = ex
                    for jt in range(ns):
                        # weave an oproj unit in every 4 j-tiles: when the
                        # exp backlog paces the scores, this keeps PE busy
                        if jt in (4, 8, 12) and ofill:
                            o_unit()
                        jsl = slice(jt * 128, (jt + 1) * 128)
                        sc_ps = ps2s.tile([128, sw], f32, tag="scores")
                        for half in range(2):
                            qsl = slice(ic * sw + half * sc,
                                        ic * sw + (half + 1) * sc)
                            nc.tensor.matmul(
                                sc_ps[:, half * sc:(half + 1) * sc],
                                kT_sb[:, h, jsl], qT_sb[:, h, qsl],
                                start=True, stop=True)
                        nc.scalar.activation(
                            ex[:, jt, :], sc_ps[:],
                            mybir.ActivationFunctionType.Exp,
                            bias=maskb_sb[:, jt:jt + 1], scale=1.0,
                        )
                    # denominator part 1: running j-tile sum into one tile
                    # (DVE, paced by the exps as they land)
                    u = treep.tile([128, sw], bf16, tag="tree")
                    nc.vector.tensor_add(u[:], ex[:, 0, :], ex[:, 1, :])
                    for a in range(2, ns):
                        nc.vector.tensor_add(u[:], u[:], ex[:, a, :])
                    roots[i] = u

                def item_P(i):
                    ic, h = divmod(i, nh_loc)
                    hsl = slice(h * d, (h + 1) * d)
                    ex = exs.pop(i)
                    root = roots.pop(i)
                    # denominator part 2: sum the 128 j-partitions with one
                    # short ones-matmul per half.  Issued here (not in S)
                    # so the PE doesn't sit on the exp->tree dependency.
                    inv = invp.tile([128, sw], f32, tag="inv")
                    rps = []
                    for half in range(2):
                        fsl = slice(half * sc, (half + 1) * sc)
                        r_ps = psA.tile([128, sc], f32, tag="oproj",
                                         name="r_ps")
                        nc.tensor.matmul(r_ps[:], ones_sb[:, :128], root[:, fsl],
                                         start=True, stop=True)
                        rps.append((r_ps, fsl))
                    for r_ps, fsl in rps:
                        nc.vector.reciprocal_approx_fast(inv[:, fsl], r_ps[:])
                    for half in range(2):
                        fsl = slice(half * sc, (half + 1) * sc)
                        pv_ps = ps2p.tile([128, sc], f32, tag="pv")
                        for jt in range(ns):
                            nc.tensor.matmul(
                                pv_ps[:], v_sb[:, jt, hsl], ex[:, jt, fsl],
                                start=(jt == 0), stop=(jt == ns - 1),
                            )
                        asl = slice(ic * sw + half * sc,
                                    ic * sw + (half + 1) * sc)
                        nc.vector.tensor_mul(attn_sb[:, h, asl],
                                             pv_ps[:], inv[:, fsl])

                def item_V(st):
                    stl = st % (sc // 128)
                    ssl = slice(stl * 128, (stl + 1) * 128)
                    xcv = xc3
                    accv = psA.tile([128, hd], f32, tag="oproj")
                    for k in range(nk):
                        nc.tensor.matmul(
                            accv[:], xcv[:, k, ssl], wv_sb[:, k, :],
                            start=(k == 0), stop=(k == nk - 1),
                        )
                    # DVE, not ScalarE: these run right as the exp pipeline
                    # ramps, and ScalarE is the scarce engine there
                    nc.vector.tensor_copy(v_sb[:, st, :], accv[:])

                # output projection, issued as (et, half) units that fill
                # PE slack between (and inside) the S/P items
                ofill = []
                ocnt = [0]

                def o_unit():
                    ic, et, half = ofill.pop(0)
                    esl = slice(et * 128, (et + 1) * 128)
                    osl = slice(ic * sw + half * sc,
                                ic * sw + (half + 1) * sc)
                    op_ps = psA.tile([128, sc], f32, tag="oproj")
                    for ht in range(nh_loc):
                        nc.tensor.matmul(
                            op_ps[:], wo_sb[:, ht, esl],
                            attn_sb[:, ht, osl],
                            start=(ht == 0), stop=(ht == nh_loc - 1),
                        )
                    ot = outp.tile([128, sc], bf16, tag="ot")
                    # stripe-0 units run while ScalarE is saturated with exp:
                    # drain on DVE only.  Tail units alternate DVE/ScalarE.
                    ocnt[0] += 1
                    if ic == 0 or ocnt[0] % 2 == 0:
                        nc.vector.tensor_copy(ot[:], op_ps[:])
                    else:
                        nc.scalar.copy(ot[:], op_ps[:])
                    nc.sync.dma_start(outT_r[:, et, osl], ot[:])

                v3 = (nch - 1) * (sc // 128)
                # S0 leads (kT clears the rope pipe ~4.5us in; scores jt0
                # waits briefly) so the ScalarE exp pipeline fills ASAP; the
                # scalar-free V items fill the exp-paced region instead of
                # the start, where ScalarE is idle anyway.
                # S0 absolutely first: its scores touch chunk-3 kT only in
                # j-tiles 12-15, and the chunk-3 rope drains k-head-0 first,
                # so the exp stream starts as early as possible
                sched = [("S", 0), ("V", v3), ("S", 1), ("V", v3 + 1),
                         ("V", v3 + 2), ("V", v3 + 3),
                         ("P", 0), ("S", 2), ("P", 1),
                         ("S", 3), ("P", 2),
                         ("S", 4), ("P", 3), ("U", 4), ("S", 5), ("U", 2),
                         ("P", 4), ("U", 4), ("S", 6), ("U", 2), ("P", 5),
                         ("U", 4), ("S", 7), ("U", 2), ("P", 6), ("U", 5),
                         ("P", 7)]
                assert nst == 2 and nh_loc == 4
                for kind, arg in sched:
                    if kind == "S":
                        item_S(arg)
                    elif kind == "P":
                        item_P(arg)
                        i = arg
                        if i % nh_loc == nh_loc - 1:
                            ic = i // nh_loc
                            ofill.extend((ic, et, half) for et in range(ne)
                                         for half in range(2))
                    elif kind == "V":
                        item_V(arg)
                    else:
                        for _ in range(min(arg, len(ofill))):
                            o_unit()
                while ofill:
                    o_unit()

    nc.compile()
    return nc


def _get_nc(s=S, dmodel=D, nh_loc=NH_LOC):
    key = (s, dmodel, nh_loc)
    if key not in _NC_CACHE:
        _NC_CACHE[key] = _build_nc(s, dmodel, nh_loc)
    return _NC_CACHE[key]


def _rope_tables(s, d, dtype=np.float32):
    inv_freq = 1.0 / (ROPE_THETA ** (np.arange(0, d, 2, dtype=np.float64) / d))
    pos = np.arange(s, dtype=np.float64)
    freqs = pos[:, None] * inv_freq[None, :]            # [s, d/2]
    emb = np.concatenate([freqs, freqs], axis=-1)       # [s, d]
    return np.cos(emb).astype(dtype), np.sin(emb).astype(dtype)


def make_in_maps(hidden_states, sequence_mask, Wqkv, Wo,
                 s=S, b=B, dmodel=D, nh_tot=N_HEADS, nh_loc=NH_LOC, d=DQK):
    bf = ml_dtypes.bfloat16
    cos, sin = _rope_tables(s, d)
    cosT = np.ascontiguousarray(cos.T).astype(bf)       # [d, s]
    sinT = np.ascontiguousarray(sin.T)                  # [d, s] f32
    ssh = sinT.copy()
    ssh[d // 2:] = -ssh[d // 2:]
    sshT = ssh.astype(bf)
    scale = 1.0 / np.sqrt(np.float32(d))

    in_maps = []
    cores_per_batch = N_CORES // b
    for c in range(N_CORES):
        bi = c // cores_per_batch
        g = c % cores_per_batch
        h0 = g * nh_loc
        hsl = slice(h0 * d, (h0 + nh_loc) * d)
        nk, sc, nch = dmodel // 128, 512, s // 512
        xb = hidden_states[:, bi, :]                    # [s, dmodel]
        # [nch, 128, nk, sc]: xTq[ch, p, k, j] = x[ch*sc+j, k*128+p]
        xTq = np.ascontiguousarray(
            xb.T.reshape(nk, 128, nch, sc).transpose(2, 1, 0, 3)).astype(bf)
        hd = nh_loc * d

        def swz_w(w):   # [dmodel, hd] -> [128, nk, hd]
            return np.ascontiguousarray(
                w.reshape(nk, 128, hd).transpose(1, 0, 2)).astype(bf)

        wq = swz_w(Wqkv[:, 0 * nh_tot * d:1 * nh_tot * d][:, hsl] * scale)
        wk = swz_w(Wqkv[:, 1 * nh_tot * d:2 * nh_tot * d][:, hsl])
        wv = swz_w(Wqkv[:, 2 * nh_tot * d:3 * nh_tot * d][:, hsl])
        # [128, nh_loc, dmodel]: wo[p, h, e] = Wo[h*128+p, e]
        wo = np.ascontiguousarray(
            Wo[hsl, :].reshape(nh_loc, 128, dmodel).transpose(1, 0, 2)
        ).astype(bf)
        bias = np.where(sequence_mask[bi] == 0, -1e30, 0.0).astype(np.float32)
        maskbT = np.ascontiguousarray(bias.reshape(s // 128, 128).T)  # [128, ns]
        in_maps.append({
            "xTq": xTq, "wq": wq, "wk": wk, "wv": wv, "wo": wo,
            "cosT": cosT, "sshT": sshT, "maskb": maskbT,
        })
    return in_maps


def kernel(hidden_states, sequence_mask, Wqkv, Wo):
    global LAST_RESULT
    from concourse.bass_utils import run_bass_kernel_spmd

    hidden_states = np.asarray(hidden_states)
    sequence_mask = np.asarray(sequence_mask)
    Wqkv = np.asarray(Wqkv)
    Wo = np.asarray(Wo)

    nc = _get_nc()
    in_maps = make_in_maps(hidden_states, sequence_mask, Wqkv, Wo)
    res = run_bass_kernel_spmd(
        nc, in_maps, list(range(N_CORES)),
        trace=bool(int(os.environ.get("KERNEL_TRACE", "0"))),
    )
    LAST_RESULT = res

    out = np.empty((S, B, D), dtype=np.float32)
    cores_per_batch = N_CORES // B
    for bi in range(B):
        acc = None
        for g in range(cores_per_batch):
            part = res.results[bi * cores_per_batch + g]["outT"]  # [D, S] bf16
            part = np.asarray(part, dtype=np.float32)
            acc = part if acc is None else acc + part
        out[:, bi, :] = acc.T
    return out
